# revision 15
# baseline (speedup 1.0000x reference)
"""Trainium2 Bass kernel for nn_CircMACBlock_v3 (8 cores, data-parallel over B).

Each core processes one batch element entirely (attention + mamba + circular
conv + router + out_proj). Everything on-device is column-major ("T" =
[channel, time]) so that depthwise convs / per-channel params are
partition-aligned and matmul outputs chain without transposes.
"""
import sys, os
sys.path.insert(0, '/opt/trn_rl_repo')

import numpy as np
import ml_dtypes

import concourse.bass as bass
import concourse.tile as tile
from concourse import mybir

F32 = mybir.dt.float32
BF16 = mybir.dt.bfloat16
AF = mybir.ActivationFunctionType
OP = mybir.AluOpType
AX = mybir.AxisListType

B, L, D = 8, 1024, 1024
H, HD = 16, 64
KC = 7
DI, DS, DC, DTR = 2048, 16, 4, 64
EPS = 1e-6
N_CORES = 8

BF = ml_dtypes.bfloat16

# ---------------------------------------------------------------------------
# wait-splitting post-pass (walrus in this container rejects >1 sync wait/inst)
import bass_rust


def _split_excess_waits(nc, max_waits=1):
    ctr = 0
    for f in nc.m.functions:
        for bb in f.blocks:
            new_insts = []
            for inst in bb.instructions:
                si = inst.sync_info
                waits = list(si.on_wait) if si and si.on_wait else []
                if len(waits) > max_waits:
                    extra, keep = waits[:-max_waits], waits[-max_waits:]
                    for i in range(0, len(extra), max_waits):
                        nop = bass_rust.InstNoOp(
                            name=f"waitsplit-{ctr}", engine=inst.engine)
                        ctr += 1
                        nop.sync_info = mybir.SyncInfo(
                            on_wait=extra[i:i + max_waits], on_update=[])
                        new_insts.append(nop)
                    si.on_wait = keep
                new_insts.append(inst)
            bb.instructions = new_insts


# ---------------------------------------------------------------------------
def build_program():
    nc = bass.Bass("TRN2", target_bir_lowering=False, debug=False,
                   num_devices=N_CORES)

    def inp(name, shape, dt):
        return nc.dram_tensor(name, list(shape), dt, kind="ExternalInput").ap()

    xT = inp("xT", [D, L], BF16)
    maskb = inp("maskb", [128, 8], F32)
    w_in = inp("w_in", [D, 4 * D], BF16)
    b_q = inp("b_q", [128, 8], F32)
    b_k = inp("b_k", [128, 8], F32)
    b_base = inp("b_base", [128, 8], F32)
    b_v_row = inp("b_v_row", [1, D], BF16)
    relb = inp("relb", [L, L], BF16)
    eye16 = inp("eye16", [128, 128], BF16)
    ones16 = inp("ones16", [1, 128], BF16)
    onesf = inp("onesf", [1, 128], F32)
    onescol16 = inp("onescol16", [128, 1], BF16)
    m_in = inp("m_in", [D, 2 * DI], BF16)
    convw = inp("convw", [128, 16 * DC], F32)
    convb = inp("convb", [128, 16], F32)
    mxw = inp("mxw", [DI, 128], F32)
    mdtw = inp("mdtw", [DTR, DI], F32)
    mdtb = inp("mdtb", [128, 16], F32)
    Acol = inp("Acol", [128, 16 * DS], F32)
    mDcol = inp("mDcol", [128, 16], F32)
    mout = inp("mout", [DI, D], BF16)
    cnnw = inp("cnnw", [128, 8 * KC], F32)
    cnnb = inp("cnnb", [128, 8], F32)
    nwa = inp("nwa", [1, D], BF16)
    nwm = inp("nwm", [1, D], BF16)
    nwc = inp("nwc", [1, D], BF16)
    rw1 = inp("rw1", [3 * D, D], BF16)
    rb1 = inp("rb1", [1, D], F32)
    rw2 = inp("rw2", [D, 3], F32)
    rb2 = inp("rb2", [1, 3], F32)
    wout = inp("wout", [D, D], BF16)
    bout_row = inp("bout_row", [1, D], BF16)
    eyef = inp("eyef", [128, 128], F32)
    epsc = inp("epsc", [1, 1], F32)

    out = nc.dram_tensor("out", [L, D], F32, kind="ExternalOutput").ap()

    ucT_d = nc.dram_tensor("ucT_d", [DI, L], F32).ap()
    sz_d = nc.dram_tensor("sz_d", [DI, L], BF16).ap()
    attn_d = nc.dram_tensor("attn_d", [D, L], BF16).ap()
    cnn_d = nc.dram_tensor("cnn_d", [D, L], BF16).ap()

    with tile.TileContext(nc) as tc:
        import contextlib
        with contextlib.ExitStack() as ctx:
            const = ctx.enter_context(tc.tile_pool(name="const", bufs=1))
            wbuf = ctx.enter_context(tc.tile_pool(name="wbuf", bufs=4))
            work = ctx.enter_context(tc.tile_pool(name="work", bufs=2))
            psum = ctx.enter_context(tc.tile_pool(name="psum", bufs=4, space="PSUM"))

            def load(name, ap_dram, shape, dt, pool=const):
                t = pool.tile(list(shape), dt, name=name)
                nc.sync.dma_start(t[:], ap_dram[:])
                return t

            eye_s = load("eye_s", eye16, [128, 128], BF16)
            eyef_s = load("eyef_s", eyef, [128, 128], F32)
            ones_s = load("ones_s", ones16, [1, 128], BF16)
            onesf_s = load("onesf_s", onesf, [1, 128], F32)
            onescol_s = load("onescol_s", onescol16, [128, 1], BF16)
            maskb_s = load("maskb_s", maskb, [128, 8], F32)
            bq_s = load("bq_s", b_q, [128, 8], F32)
            bk_s = load("bk_s", b_k, [128, 8], F32)
            bbase_s = load("bbase_s", b_base, [128, 8], F32)
            convw_s = load("convw_s", convw, [128, 16 * DC], F32)
            convb_s = load("convb_s", convb, [128, 16], F32)
            mdtb_s = load("mdtb_s", mdtb, [128, 16], F32)
            Acol_s = load("Acol_s", Acol, [128, 16 * DS], F32)
            mDcol_s = load("mDcol_s", mDcol, [128, 16], F32)
            cnnw_s = load("cnnw_s", cnnw, [128, 8 * KC], F32)
            cnnb_s = load("cnnb_s", cnnb, [128, 8], F32)
            rb2_s = load("rb2_s", rb2, [1, 3], F32)
            bout_s = load("bout_s", bout_row, [1, D], BF16)
            eps_s = load("eps_s", epsc, [1, 1], F32)
            meanbuf = const.tile([128, 24], F32, name="meanbuf")
            gcol = const.tile([128, 3], F32, name="gcol")
            mdtw_s = []
            for i in range(16):
                t = const.tile([DTR, 128], F32, name=f"mdtw{i}")
                nc.sync.dma_start(t[:], mdtw[:, 128 * i:128 * (i + 1)])
                mdtw_s.append(t)
            mxw_s = []
            for i in range(16):
                t = const.tile([128, 128], F32, name=f"mxw{i}")
                nc.sync.dma_start(t[:], mxw[128 * i:128 * (i + 1), :])
                mxw_s.append(t)
            rw2_t = []
            for i in range(8):
                t = const.tile([128, 3], F32, name=f"rw2{i}")
                nc.sync.dma_start(t[:], rw2[128 * i:128 * (i + 1), :])
                rw2_t.append(t)
            dtr_s = const.tile([DTR, L], F32, name="dtr_s")
            B16 = const.tile([DS, L], BF16, name="B16")
            C16 = const.tile([DS, L], BF16, name="C16")

            # pools in LIFO bracket order
            cm_mam = tc.tile_pool(name="pmam", bufs=1); pmam = cm_mam.__enter__()
            cm_base = tc.tile_pool(name="pbase", bufs=1); pbase = cm_base.__enter__()
            cm_qkv = tc.tile_pool(name="pqkv", bufs=1); pqkv = cm_qkv.__enter__()
            cm_x = tc.tile_pool(name="px", bufs=1); px = cm_x.__enter__()

            bvrow_s = load("bvrow_s", b_v_row, [1, D], BF16, pool=px)
            xT_t = []
            for i in range(8):
                t = px.tile([128, L], BF16, name=f"xTs{i}")
                nc.sync.dma_start(t[:], xT[128 * i:128 * (i + 1), :])
                xT_t.append(t)

            # ================= P1: in_proj ==================================
            qT_t, kT_t, baseT_t = [], [], []
            for blk, tiles, bias_s, nm, pool_sel in (
                    (0, qT_t, bq_s, "qT", None), (1, kT_t, bk_s, "kT", None),
                    (3, baseT_t, bbase_s, "baT", "base")):
                pl = pbase if pool_sel else pqkv
                for ct in range(8):
                    dst = pl.tile([128, L], BF16, name=f"{nm}{ct}")
                    for qc in range(2):
                        ps = psum.tile([128, 512], F32, name="ps", tag="ps")
                        for dc in range(8):
                            w = wbuf.tile([128, 128], BF16, name="wA", tag="wA")
                            nc.sync.dma_start(
                                w[:], w_in[128 * dc:128 * (dc + 1),
                                           1024 * blk + 128 * ct:
                                           1024 * blk + 128 * (ct + 1)])
                            nc.tensor.matmul(
                                ps[:], w[:],
                                xT_t[dc][:, 512 * qc:512 * (qc + 1)],
                                start=(dc == 0), stop=(dc == 7))
                        nc.scalar.activation(dst[:, 512 * qc:512 * (qc + 1)],
                                             ps[:], AF.Identity,
                                             bias=bias_s[:, ct:ct + 1])
                    tiles.append(dst)

            v_t = []
            for tt_ in range(8):
                dst = pqkv.tile([128, H * (HD + 1)], BF16, name=f"vpad{tt_}")
                ones_ap = dst.rearrange("p (h c) -> p h c", h=H)[:, :, HD:HD + 1]
                nc.vector.memset(ones_ap, 1.0)
                for dc2 in range(2):
                    ps = psum.tile([128, 512], F32, name="ps", tag="ps")
                    for dc in range(8):
                        w = wbuf.tile([128, 512], BF16, name="wB", tag="wB")
                        nc.sync.dma_start(
                            w[:], w_in[128 * dc:128 * (dc + 1),
                                       2048 + 512 * dc2:2048 + 512 * (dc2 + 1)])
                        nc.tensor.matmul(ps[:],
                                         xT_t[dc][:, 128 * tt_:128 * (tt_ + 1)],
                                         w[:], start=(dc == 0), stop=False)
                    nc.tensor.matmul(ps[:], ones_s[:],
                                     bvrow_s[:, 512 * dc2:512 * (dc2 + 1)],
                                     start=False, stop=True)
                    dstap = dst.rearrange("p (h c) -> p h c", h=H)[
                        :, 8 * dc2:8 * (dc2 + 1), 0:HD]
                    nc.scalar.copy(dstap, ps.rearrange("p (h c) -> p h c", h=8))
                v_t.append(dst)

            relb_t = []
            for i in range(8):
                t = pqkv.tile([128, L], BF16, name=f"relb{i}")
                nc.sync.dma_start(t[:], relb[128 * i:128 * (i + 1), :])
                relb_t.append(t)

            # ================= P2: attention (spilled to DRAM) ==============
            cm_x.__exit__(None, None, None)
            cm_exp = tc.tile_pool(name="pexp", bufs=2); pexp = cm_exp.__enter__()
            cm_av = tc.tile_pool(name="pav", bufs=2, space="PSUM"); pav = cm_av.__enter__()
            for h in range(H):
                ktile, koff = (64 * h) // 128, (64 * h) % 128
                expS = [pexp.tile([128, L], BF16, name=f"expS{_kt}")
                        for _kt in range(8)]
                for kt in range(8):
                    for qc in range(2):
                        ps = psum.tile([128, 512], F32, name="ps", tag="ps")
                        nc.tensor.matmul(ps[:], eye_s[:],
                                         relb_t[kt][:, 512 * qc:512 * (qc + 1)],
                                         start=True, stop=False)
                        nc.tensor.matmul(
                            ps[:],
                            kT_t[ktile][koff:koff + 64, 128 * kt:128 * (kt + 1)],
                            qT_t[ktile][koff:koff + 64, 512 * qc:512 * (qc + 1)],
                            start=False, stop=True)
                        nc.scalar.activation(
                            expS[kt][:, 512 * qc:512 * (qc + 1)], ps[:],
                            AF.Exp, bias=maskb_s[:, kt:kt + 1], scale=0.125)
                for qc in range(2):
                    av = pav.tile([65, 512], F32, name="p2av")
                    for kt in range(8):
                        nc.tensor.matmul(av[:], v_t[kt][:, 65 * h:65 * h + 65],
                                         expS[kt][:, 512 * qc:512 * (qc + 1)],
                                         start=(kt == 0), stop=(kt == 7))
                    rec = pexp.tile([1, 512], F32, name="rec")
                    nc.vector.reciprocal(rec[:], av[64:65, :])
                    rec16 = pexp.tile([1, 512], BF16, name="rec16")
                    nc.scalar.copy(rec16[:], rec[:])
                    rb_ps = psum.tile([64, 512], F32, name="recb", tag="ps")
                    nc.tensor.matmul(rb_ps[:], ones_s[:, 0:64], rec16[:],
                                     start=True, stop=True)
                    avs = pexp.tile([64, 512], BF16, name="avs")
                    nc.scalar.copy(avs[:], av[0:64, :])
                    att_st = pexp.tile([64, 512], BF16, name="att_st")
                    nc.vector.tensor_mul(att_st[:], avs[:], rb_ps[:])
                    nc.sync.dma_start(
                        attn_d[64 * h:64 * (h + 1), 512 * qc:512 * (qc + 1)],
                        att_st[:])
            cm_av.__exit__(None, None, None)
            cm_exp.__exit__(None, None, None)
            cm_qkv.__exit__(None, None, None)

            # ================= P3: mamba u/z/conv/x_dbl + cnn ===============
            cm_w3 = tc.tile_pool(name="pw3", bufs=2); pw3 = cm_w3.__enter__()
            cm_park = tc.tile_pool(name="ppark", bufs=1, space="PSUM")
            ppark = cm_park.__enter__()
            xdbl_ps = ppark.tile([128, 512], F32, name="xdblps0")
            xdbl_ps1 = ppark.tile([128, 512], F32, name="xdblps1")
            for i in range(16):
                ut = pw3.tile([128, L], F32, name="ut")
                for qc in range(2):
                    ps = psum.tile([128, 512], F32, name="ps", tag="ps")
                    for dc in range(8):
                        w = wbuf.tile([128, 128], BF16, name="wA", tag="wA")
                        nc.sync.dma_start(
                            w[:], m_in[128 * dc:128 * (dc + 1),
                                       128 * i:128 * (i + 1)])
                        nc.tensor.matmul(ps[:], w[:],
                                         baseT_t[dc][:, 512 * qc:512 * (qc + 1)],
                                         start=(dc == 0), stop=(dc == 7))
                    nc.scalar.copy(ut[:, 512 * qc:512 * (qc + 1)], ps[:])
                acc = pw3.tile([128, L], F32, name="convacc")
                nc.vector.tensor_scalar_mul(acc[:], ut[:],
                                            convw_s[:, 4 * i + 3:4 * i + 4])
                for j in (2, 1, 0):
                    sh = 3 - j
                    nc.vector.scalar_tensor_tensor(
                        acc[:, sh:L], ut[:, 0:L - sh],
                        convw_s[:, 4 * i + j:4 * i + j + 1],
                        acc[:, sh:L], OP.mult, OP.add)
                ub = pw3.tile([128, L], F32, name="ub")
                nc.scalar.activation(ub[:], acc[:], AF.Identity,
                                     bias=convb_s[:, i:i + 1])
                sg = pw3.tile([128, L], F32, name="sg")
                nc.scalar.activation(sg[:], ub[:], AF.Sigmoid)
                uct = pw3.tile([128, L], F32, name="uct")
                nc.vector.tensor_mul(uct[:], ub[:], sg[:])
                nc.sync.dma_start(ucT_d[128 * i:128 * (i + 1), :], uct[:])
                for qc in range(2):
                    nc.tensor.matmul(xdbl_ps[:] if qc == 0 else xdbl_ps1[:],
                                     mxw_s[i][:],
                                     uct[:, 512 * qc:512 * (qc + 1)],
                                     start=(i == 0), stop=(i == 15))
                for qc in range(2):
                    ps = psum.tile([128, 512], F32, name="ps", tag="ps")
                    for dc in range(8):
                        w = wbuf.tile([128, 128], BF16, name="wA", tag="wA")
                        nc.sync.dma_start(
                            w[:], m_in[128 * dc:128 * (dc + 1),
                                       DI + 128 * i:DI + 128 * (i + 1)])
                        nc.tensor.matmul(ps[:], w[:],
                                         baseT_t[dc][:, 512 * qc:512 * (qc + 1)],
                                         start=(dc == 0), stop=(dc == 7))
                    zsg = pw3.tile([128, 512], F32, name="zsg")
                    nc.scalar.activation(zsg[:], ps[:], AF.Sigmoid)
                    sz16t = pw3.tile([128, 512], BF16, name="sz16t")
                    nc.vector.tensor_mul(sz16t[:], ps[:], zsg[:])
                    nc.sync.dma_start(
                        sz_d[128 * i:128 * (i + 1), 512 * qc:512 * (qc + 1)],
                        sz16t[:])

            for i in range(8):
                acc = pw3.tile([128, L], F32, name="cnnacc")
                nc.vector.tensor_scalar_mul(acc[:], baseT_t[i][:],
                                            cnnw_s[:, 7 * i + 3:7 * i + 4])
                for j in range(7):
                    if j == 3:
                        continue
                    s = j - 3
                    w_ap = cnnw_s[:, 7 * i + j:7 * i + j + 1]
                    if s < 0:
                        nc.vector.scalar_tensor_tensor(
                            acc[:, -s:L], baseT_t[i][:, 0:L + s], w_ap,
                            acc[:, -s:L], OP.mult, OP.add)
                        nc.vector.scalar_tensor_tensor(
                            acc[:, 0:-s], baseT_t[i][:, L + s:L], w_ap,
                            acc[:, 0:-s], OP.mult, OP.add)
                    else:
                        nc.vector.scalar_tensor_tensor(
                            acc[:, 0:L - s], baseT_t[i][:, s:L], w_ap,
                            acc[:, 0:L - s], OP.mult, OP.add)
                        nc.vector.scalar_tensor_tensor(
                            acc[:, L - s:L], baseT_t[i][:, 0:s], w_ap,
                            acc[:, L - s:L], OP.mult, OP.add)
                cst = pw3.tile([128, L], BF16, name="cnnst")
                nc.scalar.activation(cst[:], acc[:], AF.Identity,
                                     bias=cnnb_s[:, i:i + 1])
                nc.sync.dma_start(cnn_d[128 * i:128 * (i + 1), :], cst[:])

            nc.scalar.copy(dtr_s[:, 0:512], xdbl_ps[0:DTR, :])
            nc.scalar.copy(dtr_s[:, 512:1024], xdbl_ps1[0:DTR, :])
            nc.scalar.copy(B16[:, 0:512], xdbl_ps[64:80, :])
            nc.scalar.copy(B16[:, 512:1024], xdbl_ps1[64:80, :])
            nc.scalar.copy(C16[:, 0:512], xdbl_ps[96:112, :])
            nc.scalar.copy(C16[:, 512:1024], xdbl_ps1[96:112, :])
            cm_park.__exit__(None, None, None)
            cm_w3.__exit__(None, None, None)
            cm_base.__exit__(None, None, None)

            # ================= P4: selective scan ===========================
            cm_yz = tc.tile_pool(name="pyz", bufs=1); pyz = cm_yz.__enter__()
            cm_bc = tc.tile_pool(name="pbc", bufs=1); pbc = cm_bc.__enter__()
            Bb_t, Cb_t = [], []
            for k in range(DS):
                for srct, lst, nm in ((B16, Bb_t, "Bb"), (C16, Cb_t, "Cb")):
                    row = pbc.tile([1, L], BF16, name=f"{nm}row", bufs=2)
                    nc.sync.dma_start(row[:], srct[k:k + 1, :])
                    dst = pbc.tile([128, L], BF16, name=f"{nm}{k}")
                    for qc in range(2):
                        ps = psum.tile([128, 512], F32, name="ps", tag="ps")
                        nc.tensor.matmul(ps[:], ones_s[:],
                                         row[:, 512 * qc:512 * (qc + 1)],
                                         start=True, stop=True)
                        nc.scalar.copy(dst[:, 512 * qc:512 * (qc + 1)], ps[:])
                    lst.append(dst)

            cm_sc = tc.tile_pool(name="psc", bufs=2); psc = cm_sc.__enter__()
            yz_t = []
            for i in range(16):
                dtl_ps = [psum.tile([128, 512], F32, name="dtlps", tag="ps")
                          for _ in range(2)]
                for qc in range(2):
                    nc.tensor.matmul(dtl_ps[qc][:], mdtw_s[i][:],
                                     dtr_s[:, 512 * qc:512 * (qc + 1)],
                                     start=True, stop=True)
                edt = psc.tile([128, L], F32, name="edt", bufs=1)
                for qc in range(2):
                    nc.scalar.activation(edt[:, 512 * qc:512 * (qc + 1)],
                                         dtl_ps[qc][:], AF.Exp,
                                         bias=mdtb_s[:, i:i + 1])
                dt_s = psc.tile([128, L], F32, name="dt_s", bufs=1)
                nc.scalar.activation(dt_s[:], edt[:], AF.Ln, bias=1.0)
                uc_s = psc.tile([128, L], F32, name="uc_s", bufs=1)
                nc.sync.dma_start(uc_s[:], ucT_d[128 * i:128 * (i + 1), :])
                szs = psc.tile([128, L], BF16, name="szs", bufs=1)
                nc.sync.dma_start(szs[:], sz_d[128 * i:128 * (i + 1), :])
                dtu16 = psc.tile([128, L], BF16, name="dtu16", bufs=1)
                nc.vector.tensor_mul(dtu16[:], dt_s[:], uc_s[:])
                acc_a = psc.tile([128, L], BF16, name="acc_a", bufs=1)
                acc_b = psc.tile([128, L], BF16, name="acc_b", bufs=1)
                for k in range(DS):
                    a_t = psc.tile([128, L], F32, name="a_t", bufs=1)
                    nc.scalar.activation(
                        a_t[:], dt_s[:], AF.Exp,
                        scale=Acol_s[:, 16 * i + k:16 * i + k + 1])
                    b_t = psc.tile([128, L], BF16, name="b_t")
                    nc.vector.tensor_mul(b_t[:], dtu16[:], Bb_t[k][:])
                    h_t = psc.tile([128, L], BF16, name="h_t")
                    nc.vector.tensor_tensor_scan(h_t[:], a_t[:], b_t[:], 0.0,
                                                 OP.mult, OP.add)
                    tgt = acc_a if (k % 2 == 0) else acc_b
                    if k < 2:
                        nc.vector.tensor_mul(tgt[:], h_t[:], Cb_t[k][:])
                    else:
                        hc = psc.tile([128, L], BF16, name="hc")
                        nc.vector.tensor_mul(hc[:], h_t[:], Cb_t[k][:])
                        nc.vector.tensor_add(tgt[:], tgt[:], hc[:])
                y32 = psc.tile([128, L], F32, name="y32", bufs=1)
                nc.vector.tensor_add(y32[:], acc_a[:], acc_b[:])
                nc.vector.scalar_tensor_tensor(y32[:], uc_s[:],
                                               mDcol_s[:, i:i + 1], y32[:],
                                               OP.mult, OP.add)
                yz = pyz.tile([128, L], BF16, name=f"yz{i}")
                nc.vector.tensor_mul(yz[:], y32[:], szs[:])
                yz_t.append(yz)
            cm_sc.__exit__(None, None, None)
            cm_bc.__exit__(None, None, None)

            # ================= P5: m_out -> mamba^T (SBUF) ==================
            mamba_t = []
            for ct in range(8):
                dst = pmam.tile([128, L], BF16, name=f"mamba{ct}")
                for qc in range(2):
                    ps = psum.tile([128, 512], F32, name="ps", tag="ps")
                    for dc in range(16):
                        w = wbuf.tile([128, 128], BF16, name="wA", tag="wA")
                        nc.sync.dma_start(
                            w[:], mout[128 * dc:128 * (dc + 1),
                                       128 * ct:128 * (ct + 1)])
                        nc.tensor.matmul(ps[:], w[:],
                                         yz_t[dc][:, 512 * qc:512 * (qc + 1)],
                                         start=(dc == 0), stop=(dc == 15))
                    nc.scalar.copy(dst[:, 512 * qc:512 * (qc + 1)], ps[:])
                mamba_t.append(dst)
            cm_yz.__exit__(None, None, None)

            # ================= P6: RMSNorms =================================
            cm_nrm = tc.tile_pool(name="pnrm", bufs=1); pnrm = cm_nrm.__enter__()
            nw_s = [load("nwa_s", nwa, [1, D], BF16, pool=pnrm),
                    load("nwm_s", nwm, [1, D], BF16, pool=pnrm),
                    load("nwc_s", nwc, [1, D], BF16, pool=pnrm)]
            cm_br = tc.tile_pool(name="pbr", bufs=1); pbr = cm_br.__enter__()
            cm_ss = tc.tile_pool(name="pss", bufs=1, space="PSUM"); pss = cm_ss.__enter__()
            normed = {}
            for bi, (src_kind, nm) in enumerate(
                    (("attn", "na"), ("mamba", "nm"), ("cnn", "nc"))):
                if src_kind == "mamba":
                    tiles = mamba_t
                else:
                    dram = attn_d if src_kind == "attn" else cnn_d
                    tiles = []
                    for i in range(8):
                        t = pbr.tile([128, L], BF16, name=f"br{nm}{i}")
                        nc.sync.dma_start(t[:], dram[128 * i:128 * (i + 1), :])
                        tiles.append(t)
                ss_ps = pss.tile([1, 512], F32, name=f"ssps0{nm}", tag="ssps0")
                ss_ps1 = pss.tile([1, 512], F32, name=f"ssps1{nm}", tag="ssps1")
                for i in range(8):
                    sq = pbr.tile([128, L], BF16, name="sq", bufs=2)
                    nc.scalar.activation(sq[:], tiles[i][:], AF.Square)
                    for qc in range(2):
                        nc.tensor.matmul(ss_ps[:] if qc == 0 else ss_ps1[:],
                                         onescol_s[:],
                                         sq[:, 512 * qc:512 * (qc + 1)],
                                         start=(i == 0), stop=(i == 7))
                std = pbr.tile([1, L], F32, name="std", bufs=1)
                for qc, ps in ((0, ss_ps), (1, ss_ps1)):
                    nc.scalar.activation(std[:, 512 * qc:512 * (qc + 1)], ps[:],
                                         AF.Sqrt, bias=eps_s[:], scale=1.0 / D)
                f32r = pbr.tile([1, L], F32, name="f32r", bufs=1)
                nc.vector.reciprocal(f32r[:], std[:])
                rstd = pbr.tile([1, L], BF16, name="rstd", bufs=1)
                nc.scalar.copy(rstd[:], f32r[:])
                ntiles = []
                for i in range(8):
                    nt = pnrm.tile([128, L], BF16, name=f"{nm}{i}")
                    for qc in range(2):
                        wr_ps = psum.tile([128, 512], F32, name="wrps", tag="ps")
                        nc.tensor.matmul(wr_ps[:],
                                         nw_s[bi][:, 128 * i:128 * (i + 1)],
                                         rstd[:, 512 * qc:512 * (qc + 1)],
                                         start=True, stop=True)
                        nc.vector.tensor_mul(nt[:, 512 * qc:512 * (qc + 1)],
                                             tiles[i][:, 512 * qc:512 * (qc + 1)],
                                             wr_ps[:])
                    nc.vector.reduce_sum(meanbuf[:, 8 * bi + i:8 * bi + i + 1],
                                         nt[:], axis=AX.X)
                    ntiles.append(nt)
                normed[nm] = ntiles
            cm_ss.__exit__(None, None, None)
            cm_br.__exit__(None, None, None)

            # ================= P7: router ===================================
            cm_rt = tc.tile_pool(name="prt", bufs=1); prt = cm_rt.__enter__()
            cm_r1 = tc.tile_pool(name="pr1", bufs=1, space="PSUM"); pr1 = cm_r1.__enter__()
            rb1_s = load("rb1_s", rb1, [1, D], F32, pool=prt)
            mean16 = prt.tile([128, 24], BF16, name="mean16")
            nc.scalar.activation(mean16[:], meanbuf[:], AF.Copy, scale=1.0 / L)
            r1_ps = pr1.tile([1, 512], F32, name="r1ps0")
            r1_ps1 = pr1.tile([1, 512], F32, name="r1ps1")
            for c in range(24):
                w = wbuf.tile([128, D], BF16, name="wB", tag="wB")
                nc.sync.dma_start(w[:, 0:D], rw1[128 * c:128 * (c + 1), :])
                for qc in range(2):
                    nc.tensor.matmul(r1_ps[:] if qc == 0 else r1_ps1[:],
                                     mean16[:, c:c + 1],
                                     w[:, 512 * qc:512 * (qc + 1)],
                                     start=(c == 0), stop=(c == 23))
            r1 = prt.tile([1, D], F32, name="r1")
            nc.scalar.copy(r1[:, 0:512], r1_ps[:])
            nc.scalar.copy(r1[:, 512:1024], r1_ps1[:])
            nc.vector.tensor_add(r1[:], r1[:], rb1_s[:])
            x2 = prt.tile([1, D], F32, name="gx2")
            nc.vector.tensor_mul(x2[:], r1[:], r1[:])
            x3 = prt.tile([1, D], F32, name="gx3")
            nc.vector.tensor_mul(x3[:], x2[:], r1[:])
            u_ = prt.tile([1, D], F32, name="gu")
            nc.vector.scalar_tensor_tensor(u_[:], x3[:], 0.044715, r1[:],
                                           OP.mult, OP.add)
            th = prt.tile([1, D], F32, name="gth")
            nc.scalar.activation(th[:], u_[:], AF.Tanh, scale=0.7978845608028654)
            th1 = prt.tile([1, D], F32, name="gth1")
            nc.scalar.activation(th1[:], th[:], AF.Identity, bias=1.0)
            ge = prt.tile([1, D], F32, name="ge")
            nc.vector.tensor_mul(ge[:], r1[:], th1[:])
            nc.scalar.activation(ge[:], ge[:], AF.Copy, scale=0.5)
            gecol = prt.tile([128, 8], F32, name="gecol")
            for c in range(8):
                tp = psum.tile([128, 1], F32, name="getp", tag="ps")
                nc.tensor.transpose(tp[:], ge[:, 128 * c:128 * (c + 1)],
                                    eyef_s[0:1, 0:1])
                nc.scalar.copy(gecol[:, c:c + 1], tp[:])
            r2_ps = psum.tile([1, 3], F32, name="r2ps", tag="ps")
            for c in range(8):
                nc.tensor.matmul(r2_ps[:], gecol[:, c:c + 1], rw2_t[c][:],
                                 start=(c == 0), stop=(c == 7))
            r2 = prt.tile([1, 3], F32, name="r2")
            nc.vector.tensor_add(r2[:], r2_ps[:], rb2_s[:])
            eg = prt.tile([1, 3], F32, name="eg")
            nc.scalar.activation(eg[:], r2[:], AF.Exp)
            egs = prt.tile([1, 1], F32, name="egs")
            nc.vector.reduce_sum(egs[:], eg[:], axis=AX.X)
            egr = prt.tile([1, 1], F32, name="egr")
            nc.vector.reciprocal(egr[:], egs[:])
            gates = prt.tile([1, 3], F32, name="gates")
            nc.vector.tensor_scalar_mul(gates[:], eg[:], egr[:])
            gcol_ps = psum.tile([128, 3], F32, name="gcps", tag="ps")
            nc.tensor.matmul(gcol_ps[:], onesf_s[:], gates[:],
                             start=True, stop=True)
            nc.scalar.copy(gcol[:], gcol_ps[:])
            cm_r1.__exit__(None, None, None)
            cm_rt.__exit__(None, None, None)

            # ================= P8/P9: fuse + out_proj =======================
            cm_fu = tc.tile_pool(name="pfu", bufs=1); pfu = cm_fu.__enter__()
            fused_t = []
            for i in range(8):
                f = pfu.tile([128, L], BF16, name=f"fused{i}")
                nc.vector.tensor_scalar_mul(f[:], normed["na"][i][:],
                                            gcol[:, 0:1])
                nc.vector.scalar_tensor_tensor(f[:], normed["nm"][i][:],
                                               gcol[:, 1:2], f[:],
                                               OP.mult, OP.add)
                nc.vector.scalar_tensor_tensor(f[:], normed["nc"][i][:],
                                               gcol[:, 2:3], f[:],
                                               OP.mult, OP.add)
                fused_t.append(f)
            for tt_ in range(8):
                for dc2 in range(2):
                    ps = psum.tile([128, 512], F32, name="ps", tag="ps")
                    for dc in range(8):
                        w = wbuf.tile([128, 512], BF16, name="wB", tag="wB")
                        nc.sync.dma_start(
                            w[:], wout[128 * dc:128 * (dc + 1),
                                       512 * dc2:512 * (dc2 + 1)])
                        nc.tensor.matmul(
                            ps[:], fused_t[dc][:, 128 * tt_:128 * (tt_ + 1)],
                            w[:], start=(dc == 0), stop=False)
                    nc.tensor.matmul(ps[:], ones_s[:],
                                     bout_s[:, 512 * dc2:512 * (dc2 + 1)],
                                     start=False, stop=True)
                    o = pfu.tile([128, 512], F32, name="o", bufs=2)
                    nc.scalar.copy(o[:], ps[:])
                    nc.sync.dma_start(
                        out[128 * tt_:128 * (tt_ + 1),
                            512 * dc2:512 * (dc2 + 1)], o[:])
            cm_fu.__exit__(None, None, None)
            cm_nrm.__exit__(None, None, None)
            cm_mam.__exit__(None, None, None)

    return nc


# ---------------------------------------------------------------------------
def _prep_shared(i):
    """Host-side prep of shared (per-core-identical) inputs from raw inputs."""
    s = {}
    s["w_in"] = np.asarray(i["W_in"], np.float32).astype(BF)
    b_in = np.asarray(i["b_in"], np.float32)
    s["b_q"] = b_in[0:1024].reshape(8, 128).T.copy()
    s["b_k"] = b_in[1024:2048].reshape(8, 128).T.copy()
    s["b_base"] = b_in[3072:4096].reshape(8, 128).T.copy()
    s["b_v_row"] = b_in[2048:3072].reshape(1, 1024).astype(BF)
    idx = np.arange(L)
    dmat = np.abs(idx[None, :] - idx[:, None])
    dmat = np.minimum(dmat, L - dmat).astype(np.float32)
    s["relb"] = (-8.0 * dmat).astype(BF)     # symmetric
    s["eye16"] = np.eye(128, dtype=BF)
    s["eyef"] = np.eye(128, dtype=np.float32)
    s["epsc"] = np.full((1, 1), EPS, np.float32)
    s["ones16"] = np.ones((1, 128), BF)
    s["onesf"] = np.ones((1, 128), np.float32)
    s["onescol16"] = np.ones((128, 1), BF)
    s["m_in"] = np.asarray(i["m_in_w"], np.float32).astype(BF)
    cw = np.asarray(i["m_conv_w"], np.float32).reshape(16, 128, DC)
    s["convw"] = np.ascontiguousarray(cw.transpose(1, 0, 2).reshape(128, 16 * DC))
    s["convb"] = np.asarray(i["m_conv_b"], np.float32).reshape(16, 128).T.copy()
    mxw_raw = np.asarray(i["m_x_w"], np.float32)
    mxw_pad = np.zeros((DI, 128), np.float32)
    mxw_pad[:, 0:64] = mxw_raw[:, 0:DTR]
    mxw_pad[:, 64:80] = mxw_raw[:, DTR:DTR + DS]
    mxw_pad[:, 96:112] = mxw_raw[:, DTR + DS:DTR + 2 * DS]
    s["mxw"] = mxw_pad
    s["mdtw"] = np.asarray(i["m_dt_w"], np.float32)
    s["mdtb"] = np.asarray(i["m_dt_b"], np.float32).reshape(16, 128).T.copy()
    A = -np.exp(np.asarray(i["m_A_log"], np.float32))        # [DI, DS]
    Ar = A.reshape(16, 128, DS)
    s["Acol"] = np.ascontiguousarray(Ar.transpose(1, 0, 2).reshape(128, 16 * DS))
    s["mDcol"] = np.asarray(i["m_D"], np.float32).reshape(16, 128).T.copy()
    s["mout"] = np.asarray(i["m_out_w"], np.float32).astype(BF)
    cnw = np.asarray(i["conv_w"], np.float32).reshape(8, 128, KC)
    s["cnnw"] = np.ascontiguousarray(cnw.transpose(1, 0, 2).reshape(128, 8 * KC))
    s["cnnb"] = np.asarray(i["conv_b"], np.float32).reshape(8, 128).T.copy()
    s["nwa"] = np.asarray(i["norm_attn_w"], np.float32).reshape(1, D).astype(BF)
    s["nwm"] = np.asarray(i["norm_mamba_w"], np.float32).reshape(1, D).astype(BF)
    s["nwc"] = np.asarray(i["norm_cnn_w"], np.float32).reshape(1, D).astype(BF)
    s["rw1"] = np.asarray(i["r_w1"], np.float32).astype(BF)
    s["rb1"] = np.asarray(i["r_b1"], np.float32).reshape(1, D)
    s["rw2"] = np.asarray(i["r_w2"], np.float32)
    s["rb2"] = np.asarray(i["r_b2"], np.float32).reshape(1, 3)
    s["wout"] = np.asarray(i["W_out"], np.float32).astype(BF)
    s["bout_row"] = np.asarray(i["b_out"], np.float32).reshape(1, D).astype(BF)
    return s


_CACHED = {}


def kernel(**inputs):
    from concourse.bass_utils import run_bass_kernel_spmd
    if "nc" not in _CACHED:
        nc = build_program()
        _split_excess_waits(nc)
        _CACHED["nc"] = nc
    nc = _CACHED["nc"]
    shared = _prep_shared(inputs)
    x = np.asarray(inputs["x"], np.float32)
    mask = np.asarray(inputs["mask"], np.int32)
    in_maps = []
    for c in range(N_CORES):
        m = dict(shared)
        m["xT"] = np.ascontiguousarray(x[c].T).astype(BF)
        mb = ((mask[c].astype(np.float32) - 1.0) * 1e9)
        m["maskb"] = mb.reshape(8, 128).T.copy()
        in_maps.append(m)
    res = run_bass_kernel_spmd(nc, in_maps, core_ids=list(range(N_CORES)))
    out = np.stack([res.results[c]["out"] for c in range(N_CORES)])
    return out.astype(np.float32)


# revision 16
# speedup vs baseline: 814.9893x; 814.9893x over previous
"""Trainium2 Bass kernel for nn_CircMACBlock_v3 (8 cores, data-parallel over B).

Each core processes one batch element entirely (attention + mamba + circular
conv + router + out_proj). Everything on-device is column-major ("T" =
[channel, time]) so that depthwise convs / per-channel params are
partition-aligned and matmul outputs chain without transposes.
"""
import sys, os
sys.path.insert(0, '/opt/trn_rl_repo')

import numpy as np
import ml_dtypes

import concourse.bass as bass
import concourse.tile as tile
from concourse import mybir

F32 = mybir.dt.float32
BF16 = mybir.dt.bfloat16
AF = mybir.ActivationFunctionType
OP = mybir.AluOpType
AX = mybir.AxisListType

B, L, D = 8, 1024, 1024
H, HD = 16, 64
KC = 7
DI, DS, DC, DTR = 2048, 16, 4, 64
EPS = 1e-6
N_CORES = 8

BF = ml_dtypes.bfloat16

# ---------------------------------------------------------------------------
# wait-splitting post-pass (walrus in this container rejects >1 sync wait/inst)
import bass_rust


def _split_excess_waits(nc, max_waits=1):
    ctr = 0
    for f in nc.m.functions:
        for bb in f.blocks:
            new_insts = []
            for inst in bb.instructions:
                si = inst.sync_info
                waits = list(si.on_wait) if si and si.on_wait else []
                if len(waits) > max_waits:
                    extra, keep = waits[:-max_waits], waits[-max_waits:]
                    for i in range(0, len(extra), max_waits):
                        nop = bass_rust.InstNoOp(
                            name=f"waitsplit-{ctr}", engine=inst.engine)
                        ctr += 1
                        nop.sync_info = mybir.SyncInfo(
                            on_wait=extra[i:i + max_waits], on_update=[])
                        new_insts.append(nop)
                    si.on_wait = keep
                new_insts.append(inst)
            bb.instructions = new_insts


# ---------------------------------------------------------------------------
def build_program():
    nc = bass.Bass("TRN2", target_bir_lowering=False, debug=False,
                   num_devices=N_CORES)

    def inp(name, shape, dt):
        return nc.dram_tensor(name, list(shape), dt, kind="ExternalInput").ap()

    xT = inp("xT", [D, L], BF16)
    maskb = inp("maskb", [128, 8], F32)
    w_in = inp("w_in", [D, 4 * D], BF16)
    b_q = inp("b_q", [128, 8], F32)
    b_k = inp("b_k", [128, 8], F32)
    b_base = inp("b_base", [128, 8], F32)
    b_v_row = inp("b_v_row", [1, D], BF16)
    relb = inp("relb", [L, L], BF16)
    eye16 = inp("eye16", [128, 128], BF16)
    ones16 = inp("ones16", [1, 128], BF16)
    onesf = inp("onesf", [1, 128], F32)
    onescol16 = inp("onescol16", [128, 1], BF16)
    m_in = inp("m_in", [D, 2 * DI], BF16)
    convw = inp("convw", [128, 16 * DC], F32)
    convb = inp("convb", [128, 16], F32)
    mxw = inp("mxw", [DI, 128], F32)
    mdtw = inp("mdtw", [DTR, DI], F32)
    mdtb = inp("mdtb", [128, 16], F32)
    Acol = inp("Acol", [128, 16 * DS], F32)
    mDcol = inp("mDcol", [128, 16], F32)
    mout = inp("mout", [DI, D], BF16)
    cnnw = inp("cnnw", [128, 8 * KC], F32)
    cnnb = inp("cnnb", [128, 8], F32)
    nwa = inp("nwa", [1, D], BF16)
    nwm = inp("nwm", [1, D], BF16)
    nwc = inp("nwc", [1, D], BF16)
    rw1 = inp("rw1", [3 * D, D], BF16)
    rb1 = inp("rb1", [1, D], F32)
    rw2 = inp("rw2", [D, 3], F32)
    rb2 = inp("rb2", [1, 3], F32)
    wout = inp("wout", [D, D], BF16)
    bout_row = inp("bout_row", [1, D], BF16)
    eyef = inp("eyef", [128, 128], F32)
    epsc = inp("epsc", [1, 1], F32)

    out = nc.dram_tensor("out", [L, D], F32, kind="ExternalOutput").ap()

    ucT_d = nc.dram_tensor("ucT_d", [DI, L], F32).ap()
    sz_d = nc.dram_tensor("sz_d", [DI, L], BF16).ap()
    attn_d = nc.dram_tensor("attn_d", [D, L], BF16).ap()
    cnn_d = nc.dram_tensor("cnn_d", [D, L], BF16).ap()

    with tile.TileContext(nc) as tc:
        import contextlib
        with contextlib.ExitStack() as ctx:
            const = ctx.enter_context(tc.tile_pool(name="const", bufs=1))
            wbuf = ctx.enter_context(tc.tile_pool(name="wbuf", bufs=4))
            work = ctx.enter_context(tc.tile_pool(name="work", bufs=2))
            psum = ctx.enter_context(tc.tile_pool(name="psum", bufs=6, space="PSUM"))

            def load(name, ap_dram, shape, dt, pool=const):
                t = pool.tile(list(shape), dt, name=name)
                nc.sync.dma_start(t[:], ap_dram[:])
                return t

            eye_s = load("eye_s", eye16, [128, 128], BF16)
            eyef_s = load("eyef_s", eyef, [128, 128], F32)
            ones_s = load("ones_s", ones16, [1, 128], BF16)
            onesf_s = load("onesf_s", onesf, [1, 128], F32)
            onescol_s = load("onescol_s", onescol16, [128, 1], BF16)
            maskb_s = load("maskb_s", maskb, [128, 8], F32)
            bq_s = load("bq_s", b_q, [128, 8], F32)
            bk_s = load("bk_s", b_k, [128, 8], F32)
            bbase_s = load("bbase_s", b_base, [128, 8], F32)
            convw_s = load("convw_s", convw, [128, 16 * DC], F32)
            convb_s = load("convb_s", convb, [128, 16], F32)
            mdtb_s = load("mdtb_s", mdtb, [128, 16], F32)
            Acol_s = load("Acol_s", Acol, [128, 16 * DS], F32)
            mDcol_s = load("mDcol_s", mDcol, [128, 16], F32)
            cnnw_s = load("cnnw_s", cnnw, [128, 8 * KC], F32)
            cnnb_s = load("cnnb_s", cnnb, [128, 8], F32)
            rb2_s = load("rb2_s", rb2, [1, 3], F32)
            bout_s = load("bout_s", bout_row, [1, D], BF16)
            eps_s = load("eps_s", epsc, [1, 1], F32)
            meanbuf = const.tile([128, 24], F32, name="meanbuf")
            gcol = const.tile([128, 3], F32, name="gcol")
            mdtw_s = []
            for i in range(16):
                t = const.tile([DTR, 128], F32, name=f"mdtw{i}")
                nc.sync.dma_start(t[:], mdtw[:, 128 * i:128 * (i + 1)])
                mdtw_s.append(t)
            mxw_s = []
            for i in range(16):
                t = const.tile([128, 128], F32, name=f"mxw{i}")
                nc.sync.dma_start(t[:], mxw[128 * i:128 * (i + 1), :])
                mxw_s.append(t)
            rw2_t = []
            for i in range(8):
                t = const.tile([128, 3], F32, name=f"rw2{i}")
                nc.sync.dma_start(t[:], rw2[128 * i:128 * (i + 1), :])
                rw2_t.append(t)
            dtr_s = const.tile([DTR, L], F32, name="dtr_s")
            B16 = const.tile([DS, L], BF16, name="B16")
            C16 = const.tile([DS, L], BF16, name="C16")

            # pools in LIFO bracket order
            cm_mam = tc.tile_pool(name="pmam", bufs=1); pmam = cm_mam.__enter__()
            cm_base = tc.tile_pool(name="pbase", bufs=1); pbase = cm_base.__enter__()
            cm_qkv = tc.tile_pool(name="pqkv", bufs=1); pqkv = cm_qkv.__enter__()
            cm_x = tc.tile_pool(name="px", bufs=1); px = cm_x.__enter__()

            bvrow_s = load("bvrow_s", b_v_row, [1, D], BF16, pool=px)
            xT_t = []
            for i in range(8):
                t = px.tile([128, L], BF16, name=f"xTs{i}")
                nc.sync.dma_start(t[:], xT[128 * i:128 * (i + 1), :])
                xT_t.append(t)

            # ================= P1: in_proj ==================================
            qT_t, kT_t, baseT_t = [], [], []
            for blk, tiles, bias_s, nm, pool_sel in (
                    (0, qT_t, bq_s, "qT", None), (1, kT_t, bk_s, "kT", None),
                    (3, baseT_t, bbase_s, "baT", "base")):
                pl = pbase if pool_sel else pqkv
                for ct in range(8):
                    dst = pl.tile([128, L], BF16, name=f"{nm}{ct}")
                    ps2 = [psum.tile([128, 512], F32, name="ps", tag="ps")
                           for _ in range(2)]
                    for dc in range(8):
                        w = wbuf.tile([128, 128], BF16, name="wA", tag="wA")
                        nc.sync.dma_start(
                            w[:], w_in[128 * dc:128 * (dc + 1),
                                       1024 * blk + 128 * ct:
                                       1024 * blk + 128 * (ct + 1)])
                        for qc in range(2):
                            nc.tensor.matmul(
                                ps2[qc][:], w[:],
                                xT_t[dc][:, 512 * qc:512 * (qc + 1)],
                                start=(dc == 0), stop=(dc == 7))
                    for qc in range(2):
                        nc.scalar.activation(dst[:, 512 * qc:512 * (qc + 1)],
                                             ps2[qc][:], AF.Identity,
                                             bias=bias_s[:, ct:ct + 1])
                    tiles.append(dst)

            v_t = []
            for tt_ in range(8):
                dst = pqkv.tile([128, H * (HD + 1)], BF16, name=f"vpad{tt_}")
                ones_ap = dst.rearrange("p (h c) -> p h c", h=H)[:, :, HD:HD + 1]
                nc.vector.memset(ones_ap, 1.0)
                for dc2 in range(2):
                    ps = psum.tile([128, 512], F32, name="ps", tag="ps")
                    for dc in range(8):
                        w = wbuf.tile([128, 512], BF16, name="wB", tag="wB")
                        nc.sync.dma_start(
                            w[:], w_in[128 * dc:128 * (dc + 1),
                                       2048 + 512 * dc2:2048 + 512 * (dc2 + 1)])
                        nc.tensor.matmul(ps[:],
                                         xT_t[dc][:, 128 * tt_:128 * (tt_ + 1)],
                                         w[:], start=(dc == 0), stop=False)
                    nc.tensor.matmul(ps[:], ones_s[:],
                                     bvrow_s[:, 512 * dc2:512 * (dc2 + 1)],
                                     start=False, stop=True)
                    dstap = dst.rearrange("p (h c) -> p h c", h=H)[
                        :, 8 * dc2:8 * (dc2 + 1), 0:HD]
                    nc.scalar.copy(dstap, ps.rearrange("p (h c) -> p h c", h=8))
                v_t.append(dst)

            relb_t = []
            for i in range(8):
                t = pqkv.tile([128, L], BF16, name=f"relb{i}")
                nc.sync.dma_start(t[:], relb[128 * i:128 * (i + 1), :])
                relb_t.append(t)

            # ================= P2: attention (spilled to DRAM) ==============
            cm_x.__exit__(None, None, None)
            cm_exp = tc.tile_pool(name="pexp", bufs=2); pexp = cm_exp.__enter__()
            cm_av = tc.tile_pool(name="pav", bufs=2, space="PSUM"); pav = cm_av.__enter__()
            for h in range(H):
                ktile, koff = (64 * h) // 128, (64 * h) % 128
                expS = [pexp.tile([128, L], BF16, name=f"expS{_kt}")
                        for _kt in range(8)]
                for kt in range(8):
                    sps = [psum.tile([128, 512], F32, name="ps", tag="ps")
                           for _ in range(2)]
                    for qc in range(2):
                        nc.tensor.matmul(sps[qc][:], eye_s[:],
                                         relb_t[kt][:, 512 * qc:512 * (qc + 1)],
                                         start=True, stop=False)
                    for qc in range(2):
                        nc.tensor.matmul(
                            sps[qc][:],
                            kT_t[ktile][koff:koff + 64, 128 * kt:128 * (kt + 1)],
                            qT_t[ktile][koff:koff + 64, 512 * qc:512 * (qc + 1)],
                            start=False, stop=True)
                    for qc in range(2):
                        nc.scalar.activation(
                            expS[kt][:, 512 * qc:512 * (qc + 1)], sps[qc][:],
                            AF.Exp, bias=maskb_s[:, kt:kt + 1], scale=0.125)
                for qc in range(2):
                    av = pav.tile([65, 512], F32, name="p2av")
                    for kt in range(8):
                        nc.tensor.matmul(av[:], v_t[kt][:, 65 * h:65 * h + 65],
                                         expS[kt][:, 512 * qc:512 * (qc + 1)],
                                         start=(kt == 0), stop=(kt == 7))
                    rec = pexp.tile([1, 512], F32, name="rec")
                    nc.vector.reciprocal(rec[:], av[64:65, :])
                    rec16 = pexp.tile([1, 512], BF16, name="rec16")
                    nc.scalar.copy(rec16[:], rec[:])
                    rb_ps = psum.tile([64, 512], F32, name="recb", tag="ps")
                    nc.tensor.matmul(rb_ps[:], ones_s[:, 0:64], rec16[:],
                                     start=True, stop=True)
                    avs = pexp.tile([64, 512], BF16, name="avs")
                    nc.scalar.copy(avs[:], av[0:64, :])
                    att_st = pexp.tile([64, 512], BF16, name="att_st")
                    nc.vector.tensor_mul(att_st[:], avs[:], rb_ps[:])
                    nc.sync.dma_start(
                        attn_d[64 * h:64 * (h + 1), 512 * qc:512 * (qc + 1)],
                        att_st[:])
            cm_av.__exit__(None, None, None)
            cm_exp.__exit__(None, None, None)
            cm_qkv.__exit__(None, None, None)

            # ================= P3: mamba u/z/conv/x_dbl + cnn ===============
            cm_w3 = tc.tile_pool(name="pw3", bufs=2); pw3 = cm_w3.__enter__()
            cm_park = tc.tile_pool(name="ppark", bufs=1, space="PSUM")
            ppark = cm_park.__enter__()
            xdbl_ps = ppark.tile([128, 512], F32, name="xdblps0")
            xdbl_ps1 = ppark.tile([128, 512], F32, name="xdblps1")
            for i in range(16):
                ut = pw3.tile([128, L], F32, name="ut")
                ps2 = [psum.tile([128, 512], F32, name="ps", tag="ps")
                       for _ in range(2)]
                for dc in range(8):
                    w = wbuf.tile([128, 128], BF16, name="wA", tag="wA")
                    nc.sync.dma_start(
                        w[:], m_in[128 * dc:128 * (dc + 1),
                                   128 * i:128 * (i + 1)])
                    for qc in range(2):
                        nc.tensor.matmul(ps2[qc][:], w[:],
                                         baseT_t[dc][:, 512 * qc:512 * (qc + 1)],
                                         start=(dc == 0), stop=(dc == 7))
                for qc in range(2):
                    nc.scalar.copy(ut[:, 512 * qc:512 * (qc + 1)], ps2[qc][:])
                acc = pw3.tile([128, L], F32, name="convacc")
                nc.vector.tensor_scalar_mul(acc[:], ut[:],
                                            convw_s[:, 4 * i + 3:4 * i + 4])
                for j in (2, 1, 0):
                    sh = 3 - j
                    nc.vector.scalar_tensor_tensor(
                        acc[:, sh:L], ut[:, 0:L - sh],
                        convw_s[:, 4 * i + j:4 * i + j + 1],
                        acc[:, sh:L], OP.mult, OP.add)
                ub = pw3.tile([128, L], F32, name="ub")
                nc.scalar.activation(ub[:], acc[:], AF.Identity,
                                     bias=convb_s[:, i:i + 1])
                sg = pw3.tile([128, L], F32, name="sg")
                nc.scalar.activation(sg[:], ub[:], AF.Sigmoid)
                uct = pw3.tile([128, L], F32, name="uct")
                nc.vector.tensor_mul(uct[:], ub[:], sg[:])
                nc.sync.dma_start(ucT_d[128 * i:128 * (i + 1), :], uct[:])
                for qc in range(2):
                    nc.tensor.matmul(xdbl_ps[:] if qc == 0 else xdbl_ps1[:],
                                     mxw_s[i][:],
                                     uct[:, 512 * qc:512 * (qc + 1)],
                                     start=(i == 0), stop=(i == 15))
                zps2 = [psum.tile([128, 512], F32, name="ps", tag="ps")
                        for _ in range(2)]
                for dc in range(8):
                    w = wbuf.tile([128, 128], BF16, name="wA", tag="wA")
                    nc.sync.dma_start(
                        w[:], m_in[128 * dc:128 * (dc + 1),
                                   DI + 128 * i:DI + 128 * (i + 1)])
                    for qc in range(2):
                        nc.tensor.matmul(zps2[qc][:], w[:],
                                         baseT_t[dc][:, 512 * qc:512 * (qc + 1)],
                                         start=(dc == 0), stop=(dc == 7))
                for qc in range(2):
                    zsg = pw3.tile([128, 512], F32, name="zsg")
                    nc.scalar.activation(zsg[:], zps2[qc][:], AF.Sigmoid)
                    sz16t = pw3.tile([128, 512], BF16, name="sz16t")
                    nc.vector.tensor_mul(sz16t[:], zps2[qc][:], zsg[:])
                    nc.sync.dma_start(
                        sz_d[128 * i:128 * (i + 1), 512 * qc:512 * (qc + 1)],
                        sz16t[:])

            for i in range(8):
                acc = pw3.tile([128, L], F32, name="cnnacc")
                nc.vector.tensor_scalar_mul(acc[:], baseT_t[i][:],
                                            cnnw_s[:, 7 * i + 3:7 * i + 4])
                for j in range(7):
                    if j == 3:
                        continue
                    s = j - 3
                    w_ap = cnnw_s[:, 7 * i + j:7 * i + j + 1]
                    if s < 0:
                        nc.vector.scalar_tensor_tensor(
                            acc[:, -s:L], baseT_t[i][:, 0:L + s], w_ap,
                            acc[:, -s:L], OP.mult, OP.add)
                        nc.vector.scalar_tensor_tensor(
                            acc[:, 0:-s], baseT_t[i][:, L + s:L], w_ap,
                            acc[:, 0:-s], OP.mult, OP.add)
                    else:
                        nc.vector.scalar_tensor_tensor(
                            acc[:, 0:L - s], baseT_t[i][:, s:L], w_ap,
                            acc[:, 0:L - s], OP.mult, OP.add)
                        nc.vector.scalar_tensor_tensor(
                            acc[:, L - s:L], baseT_t[i][:, 0:s], w_ap,
                            acc[:, L - s:L], OP.mult, OP.add)
                cst = pw3.tile([128, L], BF16, name="cnnst")
                nc.scalar.activation(cst[:], acc[:], AF.Identity,
                                     bias=cnnb_s[:, i:i + 1])
                nc.sync.dma_start(cnn_d[128 * i:128 * (i + 1), :], cst[:])

            nc.scalar.copy(dtr_s[:, 0:512], xdbl_ps[0:DTR, :])
            nc.scalar.copy(dtr_s[:, 512:1024], xdbl_ps1[0:DTR, :])
            nc.scalar.copy(B16[:, 0:512], xdbl_ps[64:80, :])
            nc.scalar.copy(B16[:, 512:1024], xdbl_ps1[64:80, :])
            nc.scalar.copy(C16[:, 0:512], xdbl_ps[96:112, :])
            nc.scalar.copy(C16[:, 512:1024], xdbl_ps1[96:112, :])
            cm_park.__exit__(None, None, None)
            cm_w3.__exit__(None, None, None)
            cm_base.__exit__(None, None, None)

            # ================= P4: selective scan ===========================
            cm_yz = tc.tile_pool(name="pyz", bufs=1); pyz = cm_yz.__enter__()
            cm_bc = tc.tile_pool(name="pbc", bufs=1); pbc = cm_bc.__enter__()
            Bb_t, Cb_t = [], []
            for k in range(DS):
                for srct, lst, nm in ((B16, Bb_t, "Bb"), (C16, Cb_t, "Cb")):
                    row = pbc.tile([1, L], BF16, name=f"{nm}row", bufs=2)
                    nc.sync.dma_start(row[:], srct[k:k + 1, :])
                    dst = pbc.tile([128, L], BF16, name=f"{nm}{k}")
                    for qc in range(2):
                        ps = psum.tile([128, 512], F32, name="ps", tag="ps")
                        nc.tensor.matmul(ps[:], ones_s[:],
                                         row[:, 512 * qc:512 * (qc + 1)],
                                         start=True, stop=True)
                        nc.scalar.copy(dst[:, 512 * qc:512 * (qc + 1)], ps[:])
                    lst.append(dst)

            cm_sc = tc.tile_pool(name="psc", bufs=2); psc = cm_sc.__enter__()
            yz_t = []
            for i in range(16):
                dtl_ps = [psum.tile([128, 512], F32, name="dtlps", tag="ps")
                          for _ in range(2)]
                for qc in range(2):
                    nc.tensor.matmul(dtl_ps[qc][:], mdtw_s[i][:],
                                     dtr_s[:, 512 * qc:512 * (qc + 1)],
                                     start=True, stop=True)
                edt = psc.tile([128, L], F32, name="edt", bufs=1)
                for qc in range(2):
                    nc.scalar.activation(edt[:, 512 * qc:512 * (qc + 1)],
                                         dtl_ps[qc][:], AF.Exp,
                                         bias=mdtb_s[:, i:i + 1])
                dt_s = psc.tile([128, L], F32, name="dt_s", bufs=1)
                nc.scalar.activation(dt_s[:], edt[:], AF.Ln, bias=1.0)
                uc_s = psc.tile([128, L], F32, name="uc_s", bufs=1)
                nc.sync.dma_start(uc_s[:], ucT_d[128 * i:128 * (i + 1), :])
                szs = psc.tile([128, L], BF16, name="szs", bufs=1)
                nc.sync.dma_start(szs[:], sz_d[128 * i:128 * (i + 1), :])
                dtu16 = psc.tile([128, L], BF16, name="dtu16", bufs=1)
                nc.vector.tensor_mul(dtu16[:], dt_s[:], uc_s[:])
                acc_a = psc.tile([128, L], BF16, name="acc_a", bufs=1)
                acc_b = psc.tile([128, L], BF16, name="acc_b", bufs=1)
                for k in range(DS):
                    a_t = psc.tile([128, L], F32, name="a_t", bufs=1)
                    nc.scalar.activation(
                        a_t[:], dt_s[:], AF.Exp,
                        scale=Acol_s[:, 16 * i + k:16 * i + k + 1])
                    b_t = psc.tile([128, L], BF16, name="b_t")
                    nc.vector.tensor_mul(b_t[:], dtu16[:], Bb_t[k][:])
                    h_t = psc.tile([128, L], BF16, name="h_t")
                    nc.vector.tensor_tensor_scan(h_t[:], a_t[:], b_t[:], 0.0,
                                                 OP.mult, OP.add)
                    tgt = acc_a if (k % 2 == 0) else acc_b
                    if k < 2:
                        nc.vector.tensor_mul(tgt[:], h_t[:], Cb_t[k][:])
                    else:
                        hc = psc.tile([128, L], BF16, name="hc")
                        nc.vector.tensor_mul(hc[:], h_t[:], Cb_t[k][:])
                        nc.vector.tensor_add(tgt[:], tgt[:], hc[:])
                y32 = psc.tile([128, L], F32, name="y32", bufs=1)
                nc.vector.tensor_add(y32[:], acc_a[:], acc_b[:])
                nc.vector.scalar_tensor_tensor(y32[:], uc_s[:],
                                               mDcol_s[:, i:i + 1], y32[:],
                                               OP.mult, OP.add)
                yz = pyz.tile([128, L], BF16, name=f"yz{i}")
                nc.vector.tensor_mul(yz[:], y32[:], szs[:])
                yz_t.append(yz)
            cm_sc.__exit__(None, None, None)
            cm_bc.__exit__(None, None, None)

            # ================= P5: m_out -> mamba^T (SBUF) ==================
            mamba_t = []
            for ct in range(8):
                dst = pmam.tile([128, L], BF16, name=f"mamba{ct}")
                ps2 = [psum.tile([128, 512], F32, name="ps", tag="ps")
                       for _ in range(2)]
                for dc in range(16):
                    w = wbuf.tile([128, 128], BF16, name="wA", tag="wA")
                    nc.sync.dma_start(
                        w[:], mout[128 * dc:128 * (dc + 1),
                                   128 * ct:128 * (ct + 1)])
                    for qc in range(2):
                        nc.tensor.matmul(ps2[qc][:], w[:],
                                         yz_t[dc][:, 512 * qc:512 * (qc + 1)],
                                         start=(dc == 0), stop=(dc == 15))
                for qc in range(2):
                    nc.scalar.copy(dst[:, 512 * qc:512 * (qc + 1)], ps2[qc][:])
                mamba_t.append(dst)
            cm_yz.__exit__(None, None, None)

            # ================= P6: RMSNorms =================================
            cm_nrm = tc.tile_pool(name="pnrm", bufs=1); pnrm = cm_nrm.__enter__()
            nw_s = [load("nwa_s", nwa, [1, D], BF16, pool=pnrm),
                    load("nwm_s", nwm, [1, D], BF16, pool=pnrm),
                    load("nwc_s", nwc, [1, D], BF16, pool=pnrm)]
            cm_br = tc.tile_pool(name="pbr", bufs=1); pbr = cm_br.__enter__()
            cm_ss = tc.tile_pool(name="pss", bufs=1, space="PSUM"); pss = cm_ss.__enter__()
            normed = {}
            for bi, (src_kind, nm) in enumerate(
                    (("attn", "na"), ("mamba", "nm"), ("cnn", "nc"))):
                if src_kind == "mamba":
                    tiles = mamba_t
                else:
                    dram = attn_d if src_kind == "attn" else cnn_d
                    tiles = []
                    for i in range(8):
                        t = pbr.tile([128, L], BF16, name=f"br{nm}{i}")
                        nc.sync.dma_start(t[:], dram[128 * i:128 * (i + 1), :])
                        tiles.append(t)
                ss_ps = pss.tile([1, 512], F32, name=f"ssps0{nm}", tag="ssps0")
                ss_ps1 = pss.tile([1, 512], F32, name=f"ssps1{nm}", tag="ssps1")
                for i in range(8):
                    sq = pbr.tile([128, L], BF16, name="sq", bufs=2)
                    nc.scalar.activation(sq[:], tiles[i][:], AF.Square)
                    for qc in range(2):
                        nc.tensor.matmul(ss_ps[:] if qc == 0 else ss_ps1[:],
                                         onescol_s[:],
                                         sq[:, 512 * qc:512 * (qc + 1)],
                                         start=(i == 0), stop=(i == 7))
                std = pbr.tile([1, L], F32, name="std", bufs=1)
                for qc, ps in ((0, ss_ps), (1, ss_ps1)):
                    nc.scalar.activation(std[:, 512 * qc:512 * (qc + 1)], ps[:],
                                         AF.Sqrt, bias=eps_s[:], scale=1.0 / D)
                f32r = pbr.tile([1, L], F32, name="f32r", bufs=1)
                nc.vector.reciprocal(f32r[:], std[:])
                rstd = pbr.tile([1, L], BF16, name="rstd", bufs=1)
                nc.scalar.copy(rstd[:], f32r[:])
                ntiles = []
                for i in range(8):
                    nt = pnrm.tile([128, L], BF16, name=f"{nm}{i}")
                    for qc in range(2):
                        wr_ps = psum.tile([128, 512], F32, name="wrps", tag="ps")
                        nc.tensor.matmul(wr_ps[:],
                                         nw_s[bi][:, 128 * i:128 * (i + 1)],
                                         rstd[:, 512 * qc:512 * (qc + 1)],
                                         start=True, stop=True)
                        nc.vector.tensor_mul(nt[:, 512 * qc:512 * (qc + 1)],
                                             tiles[i][:, 512 * qc:512 * (qc + 1)],
                                             wr_ps[:])
                    nc.vector.reduce_sum(meanbuf[:, 8 * bi + i:8 * bi + i + 1],
                                         nt[:], axis=AX.X)
                    ntiles.append(nt)
                normed[nm] = ntiles
            cm_ss.__exit__(None, None, None)
            cm_br.__exit__(None, None, None)

            # ================= P7: router ===================================
            cm_rt = tc.tile_pool(name="prt", bufs=1); prt = cm_rt.__enter__()
            cm_r1 = tc.tile_pool(name="pr1", bufs=1, space="PSUM"); pr1 = cm_r1.__enter__()
            rb1_s = load("rb1_s", rb1, [1, D], F32, pool=prt)
            mean16 = prt.tile([128, 24], BF16, name="mean16")
            nc.scalar.activation(mean16[:], meanbuf[:], AF.Copy, scale=1.0 / L)
            r1_ps = pr1.tile([1, 512], F32, name="r1ps0")
            r1_ps1 = pr1.tile([1, 512], F32, name="r1ps1")
            for c in range(24):
                w = wbuf.tile([128, D], BF16, name="wB", tag="wB")
                nc.sync.dma_start(w[:, 0:D], rw1[128 * c:128 * (c + 1), :])
                for qc in range(2):
                    nc.tensor.matmul(r1_ps[:] if qc == 0 else r1_ps1[:],
                                     mean16[:, c:c + 1],
                                     w[:, 512 * qc:512 * (qc + 1)],
                                     start=(c == 0), stop=(c == 23))
            r1 = prt.tile([1, D], F32, name="r1")
            nc.scalar.copy(r1[:, 0:512], r1_ps[:])
            nc.scalar.copy(r1[:, 512:1024], r1_ps1[:])
            nc.vector.tensor_add(r1[:], r1[:], rb1_s[:])
            x2 = prt.tile([1, D], F32, name="gx2")
            nc.vector.tensor_mul(x2[:], r1[:], r1[:])
            x3 = prt.tile([1, D], F32, name="gx3")
            nc.vector.tensor_mul(x3[:], x2[:], r1[:])
            u_ = prt.tile([1, D], F32, name="gu")
            nc.vector.scalar_tensor_tensor(u_[:], x3[:], 0.044715, r1[:],
                                           OP.mult, OP.add)
            th = prt.tile([1, D], F32, name="gth")
            nc.scalar.activation(th[:], u_[:], AF.Tanh, scale=0.7978845608028654)
            th1 = prt.tile([1, D], F32, name="gth1")
            nc.scalar.activation(th1[:], th[:], AF.Identity, bias=1.0)
            ge = prt.tile([1, D], F32, name="ge")
            nc.vector.tensor_mul(ge[:], r1[:], th1[:])
            nc.scalar.activation(ge[:], ge[:], AF.Copy, scale=0.5)
            gecol = prt.tile([128, 8], F32, name="gecol")
            for c in range(8):
                tp = psum.tile([128, 1], F32, name="getp", tag="ps")
                nc.tensor.transpose(tp[:], ge[:, 128 * c:128 * (c + 1)],
                                    eyef_s[0:1, 0:1])
                nc.scalar.copy(gecol[:, c:c + 1], tp[:])
            r2_ps = psum.tile([1, 3], F32, name="r2ps", tag="ps")
            for c in range(8):
                nc.tensor.matmul(r2_ps[:], gecol[:, c:c + 1], rw2_t[c][:],
                                 start=(c == 0), stop=(c == 7))
            r2 = prt.tile([1, 3], F32, name="r2")
            nc.vector.tensor_add(r2[:], r2_ps[:], rb2_s[:])
            eg = prt.tile([1, 3], F32, name="eg")
            nc.scalar.activation(eg[:], r2[:], AF.Exp)
            egs = prt.tile([1, 1], F32, name="egs")
            nc.vector.reduce_sum(egs[:], eg[:], axis=AX.X)
            egr = prt.tile([1, 1], F32, name="egr")
            nc.vector.reciprocal(egr[:], egs[:])
            gates = prt.tile([1, 3], F32, name="gates")
            nc.vector.tensor_scalar_mul(gates[:], eg[:], egr[:])
            gcol_ps = psum.tile([128, 3], F32, name="gcps", tag="ps")
            nc.tensor.matmul(gcol_ps[:], onesf_s[:], gates[:],
                             start=True, stop=True)
            nc.scalar.copy(gcol[:], gcol_ps[:])
            cm_r1.__exit__(None, None, None)
            cm_rt.__exit__(None, None, None)

            # ================= P8/P9: fuse + out_proj =======================
            cm_fu = tc.tile_pool(name="pfu", bufs=1); pfu = cm_fu.__enter__()
            fused_t = []
            for i in range(8):
                f = pfu.tile([128, L], BF16, name=f"fused{i}")
                nc.vector.tensor_scalar_mul(f[:], normed["na"][i][:],
                                            gcol[:, 0:1])
                nc.vector.scalar_tensor_tensor(f[:], normed["nm"][i][:],
                                               gcol[:, 1:2], f[:],
                                               OP.mult, OP.add)
                nc.vector.scalar_tensor_tensor(f[:], normed["nc"][i][:],
                                               gcol[:, 2:3], f[:],
                                               OP.mult, OP.add)
                fused_t.append(f)
            wout_t = []
            for dc in range(8):
                wt = pfu.tile([128, 1024], BF16, name=f"woutc{dc}")
                nc.sync.dma_start(wt[:], wout[128 * dc:128 * (dc + 1), :])
                wout_t.append(wt)
            for tt_ in range(8):
                for dc2 in range(2):
                    ps = psum.tile([128, 512], F32, name="ps", tag="ps")
                    for dc in range(8):
                        nc.tensor.matmul(
                            ps[:], fused_t[dc][:, 128 * tt_:128 * (tt_ + 1)],
                            wout_t[dc][:, 512 * dc2:512 * (dc2 + 1)],
                            start=(dc == 0), stop=False)
                    nc.tensor.matmul(ps[:], ones_s[:],
                                     bout_s[:, 512 * dc2:512 * (dc2 + 1)],
                                     start=False, stop=True)
                    o = pfu.tile([128, 512], F32, name="o", bufs=2)
                    nc.scalar.copy(o[:], ps[:])
                    nc.sync.dma_start(
                        out[128 * tt_:128 * (tt_ + 1),
                            512 * dc2:512 * (dc2 + 1)], o[:])
            cm_fu.__exit__(None, None, None)
            cm_nrm.__exit__(None, None, None)
            cm_mam.__exit__(None, None, None)

    return nc


# ---------------------------------------------------------------------------
def _prep_shared(i):
    """Host-side prep of shared (per-core-identical) inputs from raw inputs."""
    s = {}
    s["w_in"] = np.asarray(i["W_in"], np.float32).astype(BF)
    b_in = np.asarray(i["b_in"], np.float32)
    s["b_q"] = b_in[0:1024].reshape(8, 128).T.copy()
    s["b_k"] = b_in[1024:2048].reshape(8, 128).T.copy()
    s["b_base"] = b_in[3072:4096].reshape(8, 128).T.copy()
    s["b_v_row"] = b_in[2048:3072].reshape(1, 1024).astype(BF)
    idx = np.arange(L)
    dmat = np.abs(idx[None, :] - idx[:, None])
    dmat = np.minimum(dmat, L - dmat).astype(np.float32)
    s["relb"] = (-8.0 * dmat).astype(BF)     # symmetric
    s["eye16"] = np.eye(128, dtype=BF)
    s["eyef"] = np.eye(128, dtype=np.float32)
    s["epsc"] = np.full((1, 1), EPS, np.float32)
    s["ones16"] = np.ones((1, 128), BF)
    s["onesf"] = np.ones((1, 128), np.float32)
    s["onescol16"] = np.ones((128, 1), BF)
    s["m_in"] = np.asarray(i["m_in_w"], np.float32).astype(BF)
    cw = np.asarray(i["m_conv_w"], np.float32).reshape(16, 128, DC)
    s["convw"] = np.ascontiguousarray(cw.transpose(1, 0, 2).reshape(128, 16 * DC))
    s["convb"] = np.asarray(i["m_conv_b"], np.float32).reshape(16, 128).T.copy()
    mxw_raw = np.asarray(i["m_x_w"], np.float32)
    mxw_pad = np.zeros((DI, 128), np.float32)
    mxw_pad[:, 0:64] = mxw_raw[:, 0:DTR]
    mxw_pad[:, 64:80] = mxw_raw[:, DTR:DTR + DS]
    mxw_pad[:, 96:112] = mxw_raw[:, DTR + DS:DTR + 2 * DS]
    s["mxw"] = mxw_pad
    s["mdtw"] = np.asarray(i["m_dt_w"], np.float32)
    s["mdtb"] = np.asarray(i["m_dt_b"], np.float32).reshape(16, 128).T.copy()
    A = -np.exp(np.asarray(i["m_A_log"], np.float32))        # [DI, DS]
    Ar = A.reshape(16, 128, DS)
    s["Acol"] = np.ascontiguousarray(Ar.transpose(1, 0, 2).reshape(128, 16 * DS))
    s["mDcol"] = np.asarray(i["m_D"], np.float32).reshape(16, 128).T.copy()
    s["mout"] = np.asarray(i["m_out_w"], np.float32).astype(BF)
    cnw = np.asarray(i["conv_w"], np.float32).reshape(8, 128, KC)
    s["cnnw"] = np.ascontiguousarray(cnw.transpose(1, 0, 2).reshape(128, 8 * KC))
    s["cnnb"] = np.asarray(i["conv_b"], np.float32).reshape(8, 128).T.copy()
    s["nwa"] = np.asarray(i["norm_attn_w"], np.float32).reshape(1, D).astype(BF)
    s["nwm"] = np.asarray(i["norm_mamba_w"], np.float32).reshape(1, D).astype(BF)
    s["nwc"] = np.asarray(i["norm_cnn_w"], np.float32).reshape(1, D).astype(BF)
    s["rw1"] = np.asarray(i["r_w1"], np.float32).astype(BF)
    s["rb1"] = np.asarray(i["r_b1"], np.float32).reshape(1, D)
    s["rw2"] = np.asarray(i["r_w2"], np.float32)
    s["rb2"] = np.asarray(i["r_b2"], np.float32).reshape(1, 3)
    s["wout"] = np.asarray(i["W_out"], np.float32).astype(BF)
    s["bout_row"] = np.asarray(i["b_out"], np.float32).reshape(1, D).astype(BF)
    return s


_CACHED = {}


def kernel(**inputs):
    from concourse.bass_utils import run_bass_kernel_spmd
    if "nc" not in _CACHED:
        nc = build_program()
        _split_excess_waits(nc)
        _CACHED["nc"] = nc
    nc = _CACHED["nc"]
    shared = _prep_shared(inputs)
    x = np.asarray(inputs["x"], np.float32)
    mask = np.asarray(inputs["mask"], np.int32)
    in_maps = []
    for c in range(N_CORES):
        m = dict(shared)
        m["xT"] = np.ascontiguousarray(x[c].T).astype(BF)
        mb = ((mask[c].astype(np.float32) - 1.0) * 1e9)
        m["maskb"] = mb.reshape(8, 128).T.copy()
        in_maps.append(m)
    res = run_bass_kernel_spmd(nc, in_maps, core_ids=list(range(N_CORES)))
    out = np.stack([res.results[c]["out"] for c in range(N_CORES)])
    return out.astype(np.float32)


# revision 18
# speedup vs baseline: 858.2293x; 1.0531x over previous
"""Trainium2 Bass kernel for nn_CircMACBlock_v3 (8 cores, data-parallel over B).

Each core processes one batch element entirely (attention + mamba + circular
conv + router + out_proj). Everything on-device is column-major ("T" =
[channel, time]) so that depthwise convs / per-channel params are
partition-aligned and matmul outputs chain without transposes.
"""
import sys, os
sys.path.insert(0, '/opt/trn_rl_repo')

import numpy as np
import ml_dtypes

import concourse.bass as bass
import concourse.tile as tile
from concourse import mybir

F32 = mybir.dt.float32
BF16 = mybir.dt.bfloat16
AF = mybir.ActivationFunctionType
OP = mybir.AluOpType
AX = mybir.AxisListType

B, L, D = 8, 1024, 1024
H, HD = 16, 64
KC = 7
DI, DS, DC, DTR = 2048, 16, 4, 64
EPS = 1e-6
N_CORES = 8

BF = ml_dtypes.bfloat16

# ---------------------------------------------------------------------------
# wait-splitting post-pass (walrus in this container rejects >1 sync wait/inst)
import bass_rust


def _split_excess_waits(nc, max_waits=1):
    ctr = 0
    for f in nc.m.functions:
        for bb in f.blocks:
            new_insts = []
            for inst in bb.instructions:
                si = inst.sync_info
                waits = list(si.on_wait) if si and si.on_wait else []
                if len(waits) > max_waits:
                    extra, keep = waits[:-max_waits], waits[-max_waits:]
                    for i in range(0, len(extra), max_waits):
                        nop = bass_rust.InstNoOp(
                            name=f"waitsplit-{ctr}", engine=inst.engine)
                        ctr += 1
                        nop.sync_info = mybir.SyncInfo(
                            on_wait=extra[i:i + max_waits], on_update=[])
                        new_insts.append(nop)
                    si.on_wait = keep
                new_insts.append(inst)
            bb.instructions = new_insts


# ---------------------------------------------------------------------------
def build_program():
    nc = bass.Bass("TRN2", target_bir_lowering=False, debug=False,
                   num_devices=N_CORES)

    def inp(name, shape, dt):
        return nc.dram_tensor(name, list(shape), dt, kind="ExternalInput").ap()

    xT = inp("xT", [D, L], BF16)
    maskb = inp("maskb", [128, 8], F32)
    w_in = inp("w_in", [D, 4 * D], BF16)
    b_q = inp("b_q", [128, 8], F32)
    b_k = inp("b_k", [128, 8], F32)
    b_base = inp("b_base", [128, 8], F32)
    b_v_row = inp("b_v_row", [1, D], BF16)
    relb = inp("relb", [L, L], BF16)
    eye16 = inp("eye16", [128, 128], BF16)
    ones16 = inp("ones16", [1, 128], BF16)
    onesf = inp("onesf", [1, 128], F32)
    onescol16 = inp("onescol16", [128, 1], BF16)
    m_in = inp("m_in", [D, 2 * DI], BF16)
    convw = inp("convw", [128, 16 * DC], F32)
    convb = inp("convb", [128, 16], F32)
    mxw = inp("mxw", [DI, 128], F32)
    mdtw = inp("mdtw", [DTR, DI], F32)
    mdtb = inp("mdtb", [128, 16], F32)
    Acol = inp("Acol", [128, 16 * DS], F32)
    mDcol = inp("mDcol", [128, 16], F32)
    mout = inp("mout", [DI, D], BF16)
    cnnw = inp("cnnw", [128, 8 * KC], F32)
    cnnb = inp("cnnb", [128, 8], F32)
    nwa = inp("nwa", [1, D], BF16)
    nwm = inp("nwm", [1, D], BF16)
    nwc = inp("nwc", [1, D], BF16)
    rw1 = inp("rw1", [3 * D, D], BF16)
    rb1 = inp("rb1", [1, D], F32)
    rw2 = inp("rw2", [D, 3], F32)
    rb2 = inp("rb2", [1, 3], F32)
    wout = inp("wout", [D, D], BF16)
    bout_row = inp("bout_row", [1, D], BF16)
    eyef = inp("eyef", [128, 128], F32)
    epsc = inp("epsc", [1, 1], F32)

    out = nc.dram_tensor("out", [L, D], F32, kind="ExternalOutput").ap()

    ucT_d = nc.dram_tensor("ucT_d", [DI, L], F32).ap()
    sz_d = nc.dram_tensor("sz_d", [DI, L], BF16).ap()
    attn_d = nc.dram_tensor("attn_d", [D, L], BF16).ap()
    cnn_d = nc.dram_tensor("cnn_d", [D, L], BF16).ap()

    with tile.TileContext(nc) as tc:
        import contextlib
        with contextlib.ExitStack() as ctx:
            const = ctx.enter_context(tc.tile_pool(name="const", bufs=1))
            wbuf = ctx.enter_context(tc.tile_pool(name="wbuf", bufs=4))
            work = ctx.enter_context(tc.tile_pool(name="work", bufs=2))
            psum = ctx.enter_context(tc.tile_pool(name="psum", bufs=6, space="PSUM"))

            def load(name, ap_dram, shape, dt, pool=const):
                t = pool.tile(list(shape), dt, name=name)
                nc.sync.dma_start(t[:], ap_dram[:])
                return t

            eye_s = load("eye_s", eye16, [128, 128], BF16)
            eyef_s = load("eyef_s", eyef, [128, 128], F32)
            ones_s = load("ones_s", ones16, [1, 128], BF16)
            onesf_s = load("onesf_s", onesf, [1, 128], F32)
            onescol_s = load("onescol_s", onescol16, [128, 1], BF16)
            maskb_s = load("maskb_s", maskb, [128, 8], F32)
            bq_s = load("bq_s", b_q, [128, 8], F32)
            bk_s = load("bk_s", b_k, [128, 8], F32)
            bbase_s = load("bbase_s", b_base, [128, 8], F32)
            convw_s = load("convw_s", convw, [128, 16 * DC], F32)
            convb_s = load("convb_s", convb, [128, 16], F32)
            mdtb_s = load("mdtb_s", mdtb, [128, 16], F32)
            Acol_s = load("Acol_s", Acol, [128, 16 * DS], F32)
            mDcol_s = load("mDcol_s", mDcol, [128, 16], F32)
            cnnw_s = load("cnnw_s", cnnw, [128, 8 * KC], F32)
            cnnb_s = load("cnnb_s", cnnb, [128, 8], F32)
            rb2_s = load("rb2_s", rb2, [1, 3], F32)
            bout_s = load("bout_s", bout_row, [1, D], BF16)
            eps_s = load("eps_s", epsc, [1, 1], F32)
            meanbuf = const.tile([128, 24], F32, name="meanbuf")
            gcol = const.tile([128, 3], F32, name="gcol")
            mdtw_s = []
            for i in range(16):
                t = const.tile([DTR, 128], F32, name=f"mdtw{i}")
                nc.sync.dma_start(t[:], mdtw[:, 128 * i:128 * (i + 1)])
                mdtw_s.append(t)
            mxw_s = []
            for i in range(16):
                t = const.tile([128, 128], F32, name=f"mxw{i}")
                nc.sync.dma_start(t[:], mxw[128 * i:128 * (i + 1), :])
                mxw_s.append(t)
            rw2_t = []
            for i in range(8):
                t = const.tile([128, 3], F32, name=f"rw2{i}")
                nc.sync.dma_start(t[:], rw2[128 * i:128 * (i + 1), :])
                rw2_t.append(t)
            dtr_s = const.tile([DTR, L], F32, name="dtr_s")
            B16 = const.tile([DS, L], BF16, name="B16")
            C16 = const.tile([DS, L], BF16, name="C16")

            # pools in LIFO bracket order
            cm_mam = tc.tile_pool(name="pmam", bufs=1); pmam = cm_mam.__enter__()
            cm_base = tc.tile_pool(name="pbase", bufs=1); pbase = cm_base.__enter__()
            cm_qkv = tc.tile_pool(name="pqkv", bufs=1); pqkv = cm_qkv.__enter__()
            cm_x = tc.tile_pool(name="px", bufs=1); px = cm_x.__enter__()

            bvrow_s = load("bvrow_s", b_v_row, [1, D], BF16, pool=px)
            xT_t = []
            for i in range(8):
                t = px.tile([128, L], BF16, name=f"xTs{i}")
                nc.sync.dma_start(t[:], xT[128 * i:128 * (i + 1), :])
                xT_t.append(t)

            # ================= P1: in_proj ==================================
            qT_t, kT_t, baseT_t = [], [], []
            for blk, tiles, bias_s, nm, pool_sel in (
                    (0, qT_t, bq_s, "qT", None), (1, kT_t, bk_s, "kT", None),
                    (3, baseT_t, bbase_s, "baT", "base")):
                pl = pbase if pool_sel else pqkv
                for ct in range(8):
                    dst = pl.tile([128, L], BF16, name=f"{nm}{ct}")
                    ps2 = [psum.tile([128, 512], F32, name="ps", tag="ps")
                           for _ in range(2)]
                    for dc in range(8):
                        w = wbuf.tile([128, 128], BF16, name="wA", tag="wA")
                        nc.sync.dma_start(
                            w[:], w_in[128 * dc:128 * (dc + 1),
                                       1024 * blk + 128 * ct:
                                       1024 * blk + 128 * (ct + 1)])
                        for qc in range(2):
                            nc.tensor.matmul(
                                ps2[qc][:], w[:],
                                xT_t[dc][:, 512 * qc:512 * (qc + 1)],
                                start=(dc == 0), stop=(dc == 7))
                    for qc in range(2):
                        nc.scalar.activation(dst[:, 512 * qc:512 * (qc + 1)],
                                             ps2[qc][:], AF.Identity,
                                             bias=bias_s[:, ct:ct + 1])
                    tiles.append(dst)

            v_t = []
            for tt_ in range(8):
                dst = pqkv.tile([128, H * (HD + 1)], BF16, name=f"vpad{tt_}")
                ones_ap = dst.rearrange("p (h c) -> p h c", h=H)[:, :, HD:HD + 1]
                nc.vector.memset(ones_ap, 1.0)
                for dc2 in range(2):
                    ps = psum.tile([128, 512], F32, name="ps", tag="ps")
                    for dc in range(8):
                        w = wbuf.tile([128, 512], BF16, name="wB", tag="wB")
                        nc.sync.dma_start(
                            w[:], w_in[128 * dc:128 * (dc + 1),
                                       2048 + 512 * dc2:2048 + 512 * (dc2 + 1)])
                        nc.tensor.matmul(ps[:],
                                         xT_t[dc][:, 128 * tt_:128 * (tt_ + 1)],
                                         w[:], start=(dc == 0), stop=False)
                    nc.tensor.matmul(ps[:], ones_s[:],
                                     bvrow_s[:, 512 * dc2:512 * (dc2 + 1)],
                                     start=False, stop=True)
                    dstap = dst.rearrange("p (h c) -> p h c", h=H)[
                        :, 8 * dc2:8 * (dc2 + 1), 0:HD]
                    nc.scalar.copy(dstap, ps.rearrange("p (h c) -> p h c", h=8))
                v_t.append(dst)

            relb_t = []
            for i in range(8):
                t = pqkv.tile([128, L], BF16, name=f"relb{i}")
                nc.sync.dma_start(t[:], relb[128 * i:128 * (i + 1), :])
                relb_t.append(t)

            # ================= P2: attention (spilled to DRAM) ==============
            cm_x.__exit__(None, None, None)
            cm_exp = tc.tile_pool(name="pexp", bufs=2); pexp = cm_exp.__enter__()
            cm_av = tc.tile_pool(name="pav", bufs=2, space="PSUM"); pav = cm_av.__enter__()
            for h in range(H):
                ktile, koff = (64 * h) // 128, (64 * h) % 128
                expS = [pexp.tile([128, L], BF16, name=f"expS{_kt}")
                        for _kt in range(8)]
                for kt in range(8):
                    sps = [psum.tile([128, 512], F32, name="ps", tag="ps")
                           for _ in range(2)]
                    for qc in range(2):
                        nc.tensor.matmul(sps[qc][:], eye_s[:],
                                         relb_t[kt][:, 512 * qc:512 * (qc + 1)],
                                         start=True, stop=False)
                    for qc in range(2):
                        nc.tensor.matmul(
                            sps[qc][:],
                            kT_t[ktile][koff:koff + 64, 128 * kt:128 * (kt + 1)],
                            qT_t[ktile][koff:koff + 64, 512 * qc:512 * (qc + 1)],
                            start=False, stop=True)
                    for qc in range(2):
                        nc.scalar.activation(
                            expS[kt][:, 512 * qc:512 * (qc + 1)], sps[qc][:],
                            AF.Exp, bias=maskb_s[:, kt:kt + 1], scale=0.125)
                for qc in range(2):
                    av = pav.tile([65, 512], F32, name="p2av")
                    for kt in range(8):
                        nc.tensor.matmul(av[:], v_t[kt][:, 65 * h:65 * h + 65],
                                         expS[kt][:, 512 * qc:512 * (qc + 1)],
                                         start=(kt == 0), stop=(kt == 7))
                    rec = pexp.tile([1, 512], F32, name="rec")
                    nc.vector.reciprocal(rec[:], av[64:65, :])
                    rec16 = pexp.tile([1, 512], BF16, name="rec16")
                    nc.scalar.copy(rec16[:], rec[:])
                    rb_ps = psum.tile([64, 512], F32, name="recb", tag="ps")
                    nc.tensor.matmul(rb_ps[:], ones_s[:, 0:64], rec16[:],
                                     start=True, stop=True)
                    avs = pexp.tile([64, 512], BF16, name="avs")
                    nc.scalar.copy(avs[:], av[0:64, :])
                    att_st = pexp.tile([64, 512], BF16, name="att_st")
                    nc.vector.tensor_mul(att_st[:], avs[:], rb_ps[:])
                    nc.sync.dma_start(
                        attn_d[64 * h:64 * (h + 1), 512 * qc:512 * (qc + 1)],
                        att_st[:])
            cm_av.__exit__(None, None, None)
            cm_exp.__exit__(None, None, None)
            cm_qkv.__exit__(None, None, None)

            # ================= P3: mamba u/z/conv/x_dbl + cnn ===============
            cm_w3 = tc.tile_pool(name="pw3", bufs=2); pw3 = cm_w3.__enter__()
            cm_park = tc.tile_pool(name="ppark", bufs=1, space="PSUM")
            ppark = cm_park.__enter__()
            xdbl_ps = ppark.tile([128, 512], F32, name="xdblps0")
            xdbl_ps1 = ppark.tile([128, 512], F32, name="xdblps1")
            for i in range(16):
                ut = pw3.tile([128, L], F32, name="ut")
                ps2 = [psum.tile([128, 512], F32, name="ps", tag="ps")
                       for _ in range(2)]
                for dc in range(8):
                    w = wbuf.tile([128, 128], BF16, name="wA", tag="wA")
                    nc.sync.dma_start(
                        w[:], m_in[128 * dc:128 * (dc + 1),
                                   128 * i:128 * (i + 1)])
                    for qc in range(2):
                        nc.tensor.matmul(ps2[qc][:], w[:],
                                         baseT_t[dc][:, 512 * qc:512 * (qc + 1)],
                                         start=(dc == 0), stop=(dc == 7))
                for qc in range(2):
                    nc.scalar.copy(ut[:, 512 * qc:512 * (qc + 1)], ps2[qc][:])
                acc = pw3.tile([128, L], F32, name="convacc")
                nc.vector.tensor_scalar_mul(acc[:], ut[:],
                                            convw_s[:, 4 * i + 3:4 * i + 4])
                for j in (2, 1, 0):
                    sh = 3 - j
                    nc.vector.scalar_tensor_tensor(
                        acc[:, sh:L], ut[:, 0:L - sh],
                        convw_s[:, 4 * i + j:4 * i + j + 1],
                        acc[:, sh:L], OP.mult, OP.add)
                ub = pw3.tile([128, L], F32, name="ub")
                nc.scalar.activation(ub[:], acc[:], AF.Identity,
                                     bias=convb_s[:, i:i + 1])
                sg = pw3.tile([128, L], F32, name="sg")
                nc.scalar.activation(sg[:], ub[:], AF.Sigmoid)
                uct = pw3.tile([128, L], F32, name="uct")
                nc.vector.tensor_mul(uct[:], ub[:], sg[:])
                nc.sync.dma_start(ucT_d[128 * i:128 * (i + 1), :], uct[:])
                for qc in range(2):
                    nc.tensor.matmul(xdbl_ps[:] if qc == 0 else xdbl_ps1[:],
                                     mxw_s[i][:],
                                     uct[:, 512 * qc:512 * (qc + 1)],
                                     start=(i == 0), stop=(i == 15))
                zps2 = [psum.tile([128, 512], F32, name="ps", tag="ps")
                        for _ in range(2)]
                for dc in range(8):
                    w = wbuf.tile([128, 128], BF16, name="wA", tag="wA")
                    nc.sync.dma_start(
                        w[:], m_in[128 * dc:128 * (dc + 1),
                                   DI + 128 * i:DI + 128 * (i + 1)])
                    for qc in range(2):
                        nc.tensor.matmul(zps2[qc][:], w[:],
                                         baseT_t[dc][:, 512 * qc:512 * (qc + 1)],
                                         start=(dc == 0), stop=(dc == 7))
                for qc in range(2):
                    zsg = pw3.tile([128, 512], F32, name="zsg")
                    nc.scalar.activation(zsg[:], zps2[qc][:], AF.Sigmoid)
                    sz16t = pw3.tile([128, 512], BF16, name="sz16t")
                    nc.vector.tensor_mul(sz16t[:], zps2[qc][:], zsg[:])
                    nc.sync.dma_start(
                        sz_d[128 * i:128 * (i + 1), 512 * qc:512 * (qc + 1)],
                        sz16t[:])

            for i in range(8):
                acc = pw3.tile([128, L], F32, name="cnnacc")
                nc.vector.tensor_scalar_mul(acc[:], baseT_t[i][:],
                                            cnnw_s[:, 7 * i + 3:7 * i + 4])
                for j in range(7):
                    if j == 3:
                        continue
                    s = j - 3
                    w_ap = cnnw_s[:, 7 * i + j:7 * i + j + 1]
                    if s < 0:
                        nc.vector.scalar_tensor_tensor(
                            acc[:, -s:L], baseT_t[i][:, 0:L + s], w_ap,
                            acc[:, -s:L], OP.mult, OP.add)
                        nc.vector.scalar_tensor_tensor(
                            acc[:, 0:-s], baseT_t[i][:, L + s:L], w_ap,
                            acc[:, 0:-s], OP.mult, OP.add)
                    else:
                        nc.vector.scalar_tensor_tensor(
                            acc[:, 0:L - s], baseT_t[i][:, s:L], w_ap,
                            acc[:, 0:L - s], OP.mult, OP.add)
                        nc.vector.scalar_tensor_tensor(
                            acc[:, L - s:L], baseT_t[i][:, 0:s], w_ap,
                            acc[:, L - s:L], OP.mult, OP.add)
                cst = pw3.tile([128, L], BF16, name="cnnst")
                nc.scalar.activation(cst[:], acc[:], AF.Identity,
                                     bias=cnnb_s[:, i:i + 1])
                nc.sync.dma_start(cnn_d[128 * i:128 * (i + 1), :], cst[:])

            nc.scalar.copy(dtr_s[:, 0:512], xdbl_ps[0:DTR, :])
            nc.scalar.copy(dtr_s[:, 512:1024], xdbl_ps1[0:DTR, :])
            nc.scalar.copy(B16[:, 0:512], xdbl_ps[64:80, :])
            nc.scalar.copy(B16[:, 512:1024], xdbl_ps1[64:80, :])
            nc.scalar.copy(C16[:, 0:512], xdbl_ps[96:112, :])
            nc.scalar.copy(C16[:, 512:1024], xdbl_ps1[96:112, :])
            cm_park.__exit__(None, None, None)
            cm_w3.__exit__(None, None, None)
            cm_base.__exit__(None, None, None)

            # ================= P4: selective scan ===========================
            cm_yz = tc.tile_pool(name="pyz", bufs=1); pyz = cm_yz.__enter__()
            cm_bc = tc.tile_pool(name="pbc", bufs=1); pbc = cm_bc.__enter__()
            Bb_t, Cb_t = [], []
            for k in range(DS):
                for srct, lst, nm in ((B16, Bb_t, "Bb"), (C16, Cb_t, "Cb")):
                    row = pbc.tile([1, L], BF16, name=f"{nm}row", bufs=2)
                    nc.sync.dma_start(row[:], srct[k:k + 1, :])
                    dst = pbc.tile([128, L], BF16, name=f"{nm}{k}")
                    for qc in range(2):
                        ps = psum.tile([128, 512], F32, name="ps", tag="ps")
                        nc.tensor.matmul(ps[:], ones_s[:],
                                         row[:, 512 * qc:512 * (qc + 1)],
                                         start=True, stop=True)
                        nc.scalar.copy(dst[:, 512 * qc:512 * (qc + 1)], ps[:])
                    lst.append(dst)

            cm_sc = tc.tile_pool(name="psc", bufs=2); psc = cm_sc.__enter__()
            yz_t = []
            for i in range(16):
                dtl_ps = [psum.tile([128, 512], F32, name="dtlps", tag="ps")
                          for _ in range(2)]
                for qc in range(2):
                    nc.tensor.matmul(dtl_ps[qc][:], mdtw_s[i][:],
                                     dtr_s[:, 512 * qc:512 * (qc + 1)],
                                     start=True, stop=True)
                edt = psc.tile([128, L], F32, name="edt", bufs=1)
                for qc in range(2):
                    nc.scalar.activation(edt[:, 512 * qc:512 * (qc + 1)],
                                         dtl_ps[qc][:], AF.Exp,
                                         bias=mdtb_s[:, i:i + 1])
                dt_s = psc.tile([128, L], F32, name="dt_s", bufs=1)
                nc.scalar.activation(dt_s[:], edt[:], AF.Ln, bias=1.0)
                uc_s = psc.tile([128, L], F32, name="uc_s", bufs=1)
                nc.sync.dma_start(uc_s[:], ucT_d[128 * i:128 * (i + 1), :])
                szs = psc.tile([128, L], BF16, name="szs", bufs=1)
                nc.sync.dma_start(szs[:], sz_d[128 * i:128 * (i + 1), :])
                dtu16 = psc.tile([128, L], BF16, name="dtu16", bufs=1)
                nc.vector.tensor_mul(dtu16[:], dt_s[:], uc_s[:])
                acc_a = psc.tile([128, L], BF16, name="acc_a", bufs=1)
                acc_b = psc.tile([128, L], BF16, name="acc_b", bufs=1)
                for k in range(DS):
                    a_t = psc.tile([128, L], F32, name="a_t", bufs=1)
                    nc.scalar.activation(
                        a_t[:], dt_s[:], AF.Exp,
                        scale=Acol_s[:, 16 * i + k:16 * i + k + 1])
                    b_t = psc.tile([128, L], BF16, name="b_t")
                    nc.vector.tensor_mul(b_t[:], dtu16[:], Bb_t[k][:])
                    h_t = psc.tile([128, L], BF16, name="h_t")
                    nc.vector.tensor_tensor_scan(h_t[:], a_t[:], b_t[:], 0.0,
                                                 OP.mult, OP.add)
                    tgt = acc_a if (k % 2 == 0) else acc_b
                    if k < 2:
                        nc.vector.tensor_mul(tgt[:], h_t[:], Cb_t[k][:])
                    else:
                        hc = psc.tile([128, L], BF16, name="hc")
                        nc.vector.tensor_mul(hc[:], h_t[:], Cb_t[k][:])
                        nc.vector.tensor_add(tgt[:], tgt[:], hc[:])
                y32 = psc.tile([128, L], F32, name="y32", bufs=1)
                nc.vector.tensor_add(y32[:], acc_a[:], acc_b[:])
                nc.vector.scalar_tensor_tensor(y32[:], uc_s[:],
                                               mDcol_s[:, i:i + 1], y32[:],
                                               OP.mult, OP.add)
                yz = pyz.tile([128, L], BF16, name=f"yz{i}")
                nc.vector.tensor_mul(yz[:], y32[:], szs[:])
                yz_t.append(yz)
            cm_sc.__exit__(None, None, None)
            cm_bc.__exit__(None, None, None)

            # ================= P5: m_out -> mamba^T (SBUF) ==================
            mamba_t = []
            for ct in range(8):
                dst = pmam.tile([128, L], BF16, name=f"mamba{ct}")
                ps2 = [psum.tile([128, 512], F32, name="ps", tag="ps")
                       for _ in range(2)]
                for dc in range(16):
                    w = wbuf.tile([128, 128], BF16, name="wA", tag="wA")
                    nc.sync.dma_start(
                        w[:], mout[128 * dc:128 * (dc + 1),
                                   128 * ct:128 * (ct + 1)])
                    for qc in range(2):
                        nc.tensor.matmul(ps2[qc][:], w[:],
                                         yz_t[dc][:, 512 * qc:512 * (qc + 1)],
                                         start=(dc == 0), stop=(dc == 15))
                for qc in range(2):
                    nc.scalar.copy(dst[:, 512 * qc:512 * (qc + 1)], ps2[qc][:])
                mamba_t.append(dst)
            cm_yz.__exit__(None, None, None)

            # ================= P6: RMSNorms =================================
            cm_nrm = tc.tile_pool(name="pnrm", bufs=1); pnrm = cm_nrm.__enter__()
            nw_s = [load("nwa_s", nwa, [1, D], BF16, pool=pnrm),
                    load("nwm_s", nwm, [1, D], BF16, pool=pnrm),
                    load("nwc_s", nwc, [1, D], BF16, pool=pnrm)]
            cm_br = tc.tile_pool(name="pbr", bufs=1); pbr = cm_br.__enter__()
            cm_ss = tc.tile_pool(name="pss", bufs=1, space="PSUM"); pss = cm_ss.__enter__()
            normed = {}
            for bi, (src_kind, nm) in enumerate(
                    (("attn", "na"), ("mamba", "nm"), ("cnn", "nc"))):
                if src_kind == "mamba":
                    tiles = mamba_t
                else:
                    dram = attn_d if src_kind == "attn" else cnn_d
                    tiles = []
                    for i in range(8):
                        t = pbr.tile([128, L], BF16, name=f"br{nm}{i}")
                        nc.sync.dma_start(t[:], dram[128 * i:128 * (i + 1), :])
                        tiles.append(t)
                ss_ps = pss.tile([1, 512], F32, name=f"ssps0{nm}", tag="ssps0")
                ss_ps1 = pss.tile([1, 512], F32, name=f"ssps1{nm}", tag="ssps1")
                for i in range(8):
                    sq = pbr.tile([128, L], BF16, name="sq", bufs=2)
                    nc.scalar.activation(sq[:], tiles[i][:], AF.Square)
                    for qc in range(2):
                        nc.tensor.matmul(ss_ps[:] if qc == 0 else ss_ps1[:],
                                         onescol_s[:],
                                         sq[:, 512 * qc:512 * (qc + 1)],
                                         start=(i == 0), stop=(i == 7))
                std = pbr.tile([1, L], F32, name="std", bufs=1)
                for qc, ps in ((0, ss_ps), (1, ss_ps1)):
                    nc.scalar.activation(std[:, 512 * qc:512 * (qc + 1)], ps[:],
                                         AF.Sqrt, bias=eps_s[:], scale=1.0 / D)
                f32r = pbr.tile([1, L], F32, name="f32r", bufs=1)
                nc.vector.reciprocal(f32r[:], std[:])
                rstd = pbr.tile([1, L], BF16, name="rstd", bufs=1)
                nc.scalar.copy(rstd[:], f32r[:])
                ntiles = []
                for i in range(8):
                    nt = pnrm.tile([128, L], BF16, name=f"{nm}{i}")
                    for qc in range(2):
                        wr_ps = psum.tile([128, 512], F32, name="wrps", tag="ps")
                        nc.tensor.matmul(wr_ps[:],
                                         nw_s[bi][:, 128 * i:128 * (i + 1)],
                                         rstd[:, 512 * qc:512 * (qc + 1)],
                                         start=True, stop=True)
                        nc.vector.tensor_mul(nt[:, 512 * qc:512 * (qc + 1)],
                                             tiles[i][:, 512 * qc:512 * (qc + 1)],
                                             wr_ps[:])
                    nc.vector.reduce_sum(meanbuf[:, 8 * bi + i:8 * bi + i + 1],
                                         nt[:], axis=AX.X)
                    ntiles.append(nt)
                normed[nm] = ntiles
            cm_ss.__exit__(None, None, None)
            cm_br.__exit__(None, None, None)

            # ================= P7: router ===================================
            cm_rt = tc.tile_pool(name="prt", bufs=1); prt = cm_rt.__enter__()
            cm_r1 = tc.tile_pool(name="pr1", bufs=1, space="PSUM"); pr1 = cm_r1.__enter__()
            rb1_s = load("rb1_s", rb1, [1, D], F32, pool=prt)
            mean16 = prt.tile([128, 24], BF16, name="mean16")
            nc.scalar.activation(mean16[:], meanbuf[:], AF.Copy, scale=1.0 / L)
            r1_ps = pr1.tile([1, 512], F32, name="r1ps0")
            r1_ps1 = pr1.tile([1, 512], F32, name="r1ps1")
            for c in range(24):
                w = wbuf.tile([128, D], BF16, name="wB", tag="wB")
                nc.sync.dma_start(w[:, 0:D], rw1[128 * c:128 * (c + 1), :])
                for qc in range(2):
                    nc.tensor.matmul(r1_ps[:] if qc == 0 else r1_ps1[:],
                                     mean16[:, c:c + 1],
                                     w[:, 512 * qc:512 * (qc + 1)],
                                     start=(c == 0), stop=(c == 23))
            r1 = prt.tile([1, D], F32, name="r1")
            nc.scalar.copy(r1[:, 0:512], r1_ps[:])
            nc.scalar.copy(r1[:, 512:1024], r1_ps1[:])
            nc.vector.tensor_add(r1[:], r1[:], rb1_s[:])
            x2 = prt.tile([1, D], F32, name="gx2")
            nc.vector.tensor_mul(x2[:], r1[:], r1[:])
            x3 = prt.tile([1, D], F32, name="gx3")
            nc.vector.tensor_mul(x3[:], x2[:], r1[:])
            u_ = prt.tile([1, D], F32, name="gu")
            nc.vector.scalar_tensor_tensor(u_[:], x3[:], 0.044715, r1[:],
                                           OP.mult, OP.add)
            th = prt.tile([1, D], F32, name="gth")
            nc.scalar.activation(th[:], u_[:], AF.Tanh, scale=0.7978845608028654)
            th1 = prt.tile([1, D], F32, name="gth1")
            nc.scalar.activation(th1[:], th[:], AF.Identity, bias=1.0)
            ge = prt.tile([1, D], F32, name="ge")
            nc.vector.tensor_mul(ge[:], r1[:], th1[:])
            nc.scalar.activation(ge[:], ge[:], AF.Copy, scale=0.5)
            gecol = prt.tile([128, 8], F32, name="gecol")
            for c in range(8):
                tp = psum.tile([128, 1], F32, name="getp", tag="ps")
                nc.tensor.transpose(tp[:], ge[:, 128 * c:128 * (c + 1)],
                                    eyef_s[0:1, 0:1])
                nc.scalar.copy(gecol[:, c:c + 1], tp[:])
            r2_ps = psum.tile([1, 3], F32, name="r2ps", tag="ps")
            for c in range(8):
                nc.tensor.matmul(r2_ps[:], gecol[:, c:c + 1], rw2_t[c][:],
                                 start=(c == 0), stop=(c == 7))
            r2 = prt.tile([1, 3], F32, name="r2")
            nc.vector.tensor_add(r2[:], r2_ps[:], rb2_s[:])
            eg = prt.tile([1, 3], F32, name="eg")
            nc.scalar.activation(eg[:], r2[:], AF.Exp)
            egs = prt.tile([1, 1], F32, name="egs")
            nc.vector.reduce_sum(egs[:], eg[:], axis=AX.X)
            egr = prt.tile([1, 1], F32, name="egr")
            nc.vector.reciprocal(egr[:], egs[:])
            gates = prt.tile([1, 3], F32, name="gates")
            nc.vector.tensor_scalar_mul(gates[:], eg[:], egr[:])
            gcol_ps = psum.tile([128, 3], F32, name="gcps", tag="ps")
            nc.tensor.matmul(gcol_ps[:], onesf_s[:], gates[:],
                             start=True, stop=True)
            nc.scalar.copy(gcol[:], gcol_ps[:])
            cm_r1.__exit__(None, None, None)
            cm_rt.__exit__(None, None, None)

            # ================= P8/P9: fuse + out_proj =======================
            cm_fu = tc.tile_pool(name="pfu", bufs=1); pfu = cm_fu.__enter__()
            fused_t = []
            for i in range(8):
                f = pfu.tile([128, L], BF16, name=f"fused{i}")
                nc.vector.tensor_scalar_mul(f[:], normed["na"][i][:],
                                            gcol[:, 0:1])
                nc.vector.scalar_tensor_tensor(f[:], normed["nm"][i][:],
                                               gcol[:, 1:2], f[:],
                                               OP.mult, OP.add)
                nc.vector.scalar_tensor_tensor(f[:], normed["nc"][i][:],
                                               gcol[:, 2:3], f[:],
                                               OP.mult, OP.add)
                fused_t.append(f)
            wout_t = []
            for dc in range(8):
                wt = pfu.tile([128, 1024], BF16, name=f"woutc{dc}")
                nc.sync.dma_start(wt[:], wout[128 * dc:128 * (dc + 1), :])
                wout_t.append(wt)
            for tt_ in range(8):
                for dc2 in range(2):
                    ps = psum.tile([128, 512], F32, name="ps", tag="ps")
                    for dc in range(8):
                        nc.tensor.matmul(
                            ps[:], fused_t[dc][:, 128 * tt_:128 * (tt_ + 1)],
                            wout_t[dc][:, 512 * dc2:512 * (dc2 + 1)],
                            start=(dc == 0), stop=False)
                    nc.tensor.matmul(ps[:], ones_s[:],
                                     bout_s[:, 512 * dc2:512 * (dc2 + 1)],
                                     start=False, stop=True)
                    o = pfu.tile([128, 512], F32, name="o", bufs=2)
                    nc.scalar.copy(o[:], ps[:])
                    nc.sync.dma_start(
                        out[128 * tt_:128 * (tt_ + 1),
                            512 * dc2:512 * (dc2 + 1)], o[:])
            cm_fu.__exit__(None, None, None)
            cm_nrm.__exit__(None, None, None)
            cm_mam.__exit__(None, None, None)

    return nc


# ---------------------------------------------------------------------------
def _prep_shared(i):
    """Host-side prep of shared (per-core-identical) inputs from raw inputs."""
    s = {}
    s["w_in"] = np.asarray(i["W_in"], np.float32).astype(BF)
    b_in = np.asarray(i["b_in"], np.float32)
    s["b_q"] = b_in[0:1024].reshape(8, 128).T.copy()
    s["b_k"] = b_in[1024:2048].reshape(8, 128).T.copy()
    s["b_base"] = b_in[3072:4096].reshape(8, 128).T.copy()
    s["b_v_row"] = b_in[2048:3072].reshape(1, 1024).astype(BF)
    idx = np.arange(L)
    dmat = np.abs(idx[None, :] - idx[:, None])
    dmat = np.minimum(dmat, L - dmat).astype(np.float32)
    s["relb"] = (-8.0 * dmat).astype(BF)     # symmetric
    s["eye16"] = np.eye(128, dtype=BF)
    s["eyef"] = np.eye(128, dtype=np.float32)
    s["epsc"] = np.full((1, 1), EPS, np.float32)
    s["ones16"] = np.ones((1, 128), BF)
    s["onesf"] = np.ones((1, 128), np.float32)
    s["onescol16"] = np.ones((128, 1), BF)
    s["m_in"] = np.asarray(i["m_in_w"], np.float32).astype(BF)
    cw = np.asarray(i["m_conv_w"], np.float32).reshape(16, 128, DC)
    s["convw"] = np.ascontiguousarray(cw.transpose(1, 0, 2).reshape(128, 16 * DC))
    s["convb"] = np.asarray(i["m_conv_b"], np.float32).reshape(16, 128).T.copy()
    mxw_raw = np.asarray(i["m_x_w"], np.float32)
    mxw_pad = np.zeros((DI, 128), np.float32)
    mxw_pad[:, 0:64] = mxw_raw[:, 0:DTR]
    mxw_pad[:, 64:80] = mxw_raw[:, DTR:DTR + DS]
    mxw_pad[:, 96:112] = mxw_raw[:, DTR + DS:DTR + 2 * DS]
    s["mxw"] = mxw_pad
    s["mdtw"] = np.asarray(i["m_dt_w"], np.float32)
    s["mdtb"] = np.asarray(i["m_dt_b"], np.float32).reshape(16, 128).T.copy()
    A = -np.exp(np.asarray(i["m_A_log"], np.float32))        # [DI, DS]
    Ar = A.reshape(16, 128, DS)
    s["Acol"] = np.ascontiguousarray(Ar.transpose(1, 0, 2).reshape(128, 16 * DS))
    s["mDcol"] = np.asarray(i["m_D"], np.float32).reshape(16, 128).T.copy()
    s["mout"] = np.asarray(i["m_out_w"], np.float32).astype(BF)
    cnw = np.asarray(i["conv_w"], np.float32).reshape(8, 128, KC)
    s["cnnw"] = np.ascontiguousarray(cnw.transpose(1, 0, 2).reshape(128, 8 * KC))
    s["cnnb"] = np.asarray(i["conv_b"], np.float32).reshape(8, 128).T.copy()
    s["nwa"] = np.asarray(i["norm_attn_w"], np.float32).reshape(1, D).astype(BF)
    s["nwm"] = np.asarray(i["norm_mamba_w"], np.float32).reshape(1, D).astype(BF)
    s["nwc"] = np.asarray(i["norm_cnn_w"], np.float32).reshape(1, D).astype(BF)
    s["rw1"] = np.asarray(i["r_w1"], np.float32).astype(BF)
    s["rb1"] = np.asarray(i["r_b1"], np.float32).reshape(1, D)
    s["rw2"] = np.asarray(i["r_w2"], np.float32)
    s["rb2"] = np.asarray(i["r_b2"], np.float32).reshape(1, 3)
    s["wout"] = np.asarray(i["W_out"], np.float32).astype(BF)
    s["bout_row"] = np.asarray(i["b_out"], np.float32).reshape(1, D).astype(BF)
    return s


_CACHED = {}


def kernel(**inputs):
    from concourse.bass_utils import run_bass_kernel_spmd
    if "nc" not in _CACHED:
        nc = build_program()
        _split_excess_waits(nc)
        _CACHED["nc"] = nc
    nc = _CACHED["nc"]
    shared = _prep_shared(inputs)
    x = np.asarray(inputs["x"], np.float32)
    mask = np.asarray(inputs["mask"], np.int32)
    in_maps = []
    for c in range(N_CORES):
        m = dict(shared)
        m["xT"] = np.ascontiguousarray(x[c].T).astype(BF)
        mb = ((mask[c].astype(np.float32) - 1.0) * 1e9)
        m["maskb"] = mb.reshape(8, 128).T.copy()
        in_maps.append(m)
    res = run_bass_kernel_spmd(nc, in_maps, core_ids=list(range(N_CORES)))
    out = np.stack([res.results[c]["out"] for c in range(N_CORES)])
    return out.astype(np.float32)
